# revision 1
# baseline (speedup 1.0000x reference)
"""Trainium2 Bass kernel for a 2-layer GraphConv GCN (nn_GCNN_69776038691375).

reference semantics:
    x = h.swapaxes(0,1)                       # [N, B, F]
    out_deg/in_deg from src/dst, clipped at 1
    s = out_deg**-0.5 ; d = in_deg**-0.5
    layer(x, W, b) = (segsum((x*s)[src] -> dst) * d) @ W + b
    y = relu(layer(x, W1, b1)); out = layer(y, W2, b2); return out.swapaxes(0,1)

Key identity used on device: aggregation commutes with the feature transform,
so each layer computes  agg((x*s) @ W) * d + b  — for layer 2 this shrinks the
gathered rows from 256 to 128 floats.

Distribution (8 cores): destination-node sharding. Nodes padded to
NPAD=50176 = 8 cores x 49 blocks x 128 nodes. Core c owns global blocks
[c*49, (c+1)*49). Edges are grouped by dst block; within a block they are
split into lo (src < 25088) / hi halves because dma_gather indices are int16.
Aggregation = gathered rows (dma_gather) reduced with a one-hot matrix built
on device (is_equal against a column-index matrix) via TensorE matmuls
accumulating in PSUM. In-degree falls out of the same matmuls against a ones
column; out-degree comes from an identical counting pass over src-sorted
edges. s_norm (tiny) and the layer-2 table (25.7MB) are AllGathered on-chip.
"""

import numpy as np

import concourse.bacc as bacc
import concourse.bass as bass
import concourse.mybir as mybir
import concourse.tile as tile
from concourse.bass_interp import get_hw_module
from concourse.bass_utils import run_bass_kernel_spmd

F32 = mybir.dt.float32
I16 = mybir.dt.int16

# problem sizes (hardcoded per contract)
N = 50000
E = 800000
B = 4
IN_D, HID_D, OUT_D = 64, 64, 32
NCORES = 8
PB = 49                 # blocks per core
NB = NCORES * PB        # 392 global blocks
NPAD = NB * 128         # 50176
HALF = NPAD // 2        # 25088: dma_gather int16 index limit split point
D1 = B * HID_D          # 256 floats per layer-1 table row
D2 = B * OUT_D          # 128 floats per layer-2 table row
SENT = 250              # one-hot sentinel for padded edges
SPLIT = 24              # L1-loop block index after which the first y2w AllGather fires


# ---------------------------------------------------------------- host side

def _wrap_idx(flat):
    """dma_gather index layout: idx j of a gather lives at [j%16, j//16],
    replicated across the 8 groups of 16 partitions. flat: [T, 128] int16
    (subtile-major). Returns [128, T*8]."""
    T = flat.shape[0]
    w = flat.reshape(T, 8, 16).transpose(2, 0, 1).reshape(16, T * 8)
    return np.tile(w, (8, 1)).astype(np.int16)


def _preprocess(src, dst):
    """Build per-core padded edge structures. Returns (percore, C_lo, C_hi, Sd)."""
    src = np.asarray(src).astype(np.int64)
    dst = np.asarray(dst).astype(np.int64)

    # ---- dst-sorted structure for the aggregation passes
    blk = dst >> 7
    hi = (src >= HALF).astype(np.int64)
    order = np.lexsort((src, hi, blk))
    s_src, s_dst, s_blk, s_hi = src[order], dst[order], blk[order], hi[order]
    # counts per (block, half)
    cnt = np.bincount(s_blk * 2 + s_hi, minlength=NB * 2).reshape(NB, 2)
    starts = np.concatenate([[0], np.cumsum(cnt.ravel())])[:-1].reshape(NB, 2)
    # per block-index subtile counts, max over cores (shared program shape)
    lo_sub = -(-cnt[:, 0] // 128).reshape(NCORES, PB)
    hi_sub = -(-cnt[:, 1] // 128).reshape(NCORES, PB)
    C_lo = np.maximum(lo_sub.max(axis=0), 1).astype(int)
    C_hi = hi_sub.max(axis=0).astype(int)

    # ---- src-sorted structure for the out-degree pass
    sblk = src >> 7
    order2 = np.argsort(sblk, kind="stable")
    d_src, d_sblk = src[order2], sblk[order2]
    dcnt = np.bincount(d_sblk, minlength=NB)
    dstarts = np.concatenate([[0], np.cumsum(dcnt)])[:-1]
    dsub = -(-dcnt // 128).reshape(NCORES, PB)
    Sd = np.maximum(dsub.max(axis=0), 1).astype(int)

    T_agg = int(C_lo.sum() + C_hi.sum())
    T_deg = int(Sd.sum())

    # ---- L2 structure: table is the concat of two AllGather outputs:
    # A = per-core blocks 0..SPLIT-1 (chunk SPLIT*128 rows/rank),
    # B = per-core blocks SPLIT..PB-1. Positions fit int16.
    src_c = src // (PB * 128)
    src_b = (src % (PB * 128)) >> 7
    src_p = src & 127
    in_b2 = (src_b >= SPLIT).astype(np.int64)
    pos = np.where(in_b2 == 0,
                   src_c * (SPLIT * 128) + src_b * 128 + src_p,
                   src_c * ((PB - SPLIT) * 128) + (src_b - SPLIT) * 128 + src_p)
    order3 = np.lexsort((src, in_b2, blk))
    t_pos, t_dst, t_blk, t_b2 = pos[order3], dst[order3], blk[order3], in_b2[order3]
    cnt2 = np.bincount(t_blk * 2 + t_b2, minlength=NB * 2).reshape(NB, 2)
    starts2 = np.concatenate([[0], np.cumsum(cnt2.ravel())])[:-1].reshape(NB, 2)
    a_sub = -(-cnt2[:, 0] // 128).reshape(NCORES, PB)
    b_sub = -(-cnt2[:, 1] // 128).reshape(NCORES, PB)
    C_a = np.maximum(a_sub.max(axis=0), 1).astype(int)
    C_b = b_sub.max(axis=0).astype(int)
    T_ag2 = int(C_a.sum() + C_b.sum())

    percore = []
    for c in range(NCORES):
        gsl = []  # gather indices, [T_agg, 128] int16 (relative to half)
        dsl = []  # dst-local,      [T_agg, 128] int16
        for b in range(PB):
            g = c * PB + b
            base = g * 128
            for h, C in ((0, C_lo[b]), (1, C_hi[b])):
                n = int(cnt[g, h])
                st = int(starts[g, h])
                gi = np.zeros(C * 128, np.int16)
                dl = np.full(C * 128, SENT, np.int16)
                gi[:n] = (s_src[st:st + n] - h * HALF).astype(np.int16)
                dl[:n] = (s_dst[st:st + n] - base).astype(np.int16)
                gsl.append(gi.reshape(C, 128))
                dsl.append(dl.reshape(C, 128))
        gs = np.concatenate(gsl, axis=0)
        ds = np.concatenate(dsl, axis=0)
        sl = []  # src-local for degree pass, [T_deg, 128] int16
        for b in range(PB):
            g = c * PB + b
            n = int(dcnt[g])
            st = int(dstarts[g])
            s = np.full(Sd[b] * 128, SENT, np.int16)
            s[:n] = (d_src[st:st + n] - g * 128).astype(np.int16)
            sl.append(s.reshape(Sd[b], 128))
        sv = np.concatenate(sl, axis=0)
        gsl2, dsl2 = [], []
        for b in range(PB):
            g = c * PB + b
            base = g * 128
            for h, C in ((0, C_a[b]), (1, C_b[b])):
                n = int(cnt2[g, h])
                st = int(starts2[g, h])
                gi = np.zeros(C * 128, np.int16)
                dl = np.full(C * 128, SENT, np.int16)
                gi[:n] = t_pos[st:st + n].astype(np.int16)
                dl[:n] = (t_dst[st:st + n] - base).astype(np.int16)
                gsl2.append(gi.reshape(C, 128))
                dsl2.append(dl.reshape(C, 128))
        gs2 = np.concatenate(gsl2, axis=0)
        ds2 = np.concatenate(dsl2, axis=0)
        percore.append({
            "gidx": _wrap_idx(gs),            # [128, T_agg*8]
            "dstl": np.ascontiguousarray(ds.T),  # [128, T_agg]
            "srcl": np.ascontiguousarray(sv.T),  # [128, T_deg]
            "gidx2": _wrap_idx(gs2),             # [128, T_ag2*8]
            "dstl2": np.ascontiguousarray(ds2.T),  # [128, T_ag2]
        })
    meta = dict(C_lo=C_lo.tolist(), C_hi=C_hi.tolist(), Sd=Sd.tolist(),
                C_a=C_a.tolist(), C_b=C_b.tolist(),
                T_agg=T_agg, T_deg=T_deg, T_ag2=T_ag2)
    return percore, meta


# -------------------------------------------------------------- bass program

def _build(meta, collectives=True, upto='l2'):
    C_lo, C_hi, Sd = meta["C_lo"], meta["C_hi"], meta["Sd"]
    C_a, C_b = meta["C_a"], meta["C_b"]
    T_agg, T_deg, T_ag2 = meta["T_agg"], meta["T_deg"], meta["T_ag2"]
    CMAX = max(max(C_lo[b] + C_hi[b] for b in range(PB)),
               max(C_a[b] + C_b[b] for b in range(PB)), max(Sd))
    nc = bacc.Bacc("TRN2", target_bir_lowering=False, debug=False,
                   num_devices=NCORES)

    hT = nc.dram_tensor("hT", [B, IN_D, NPAD], F32, kind="ExternalInput")
    w1 = nc.dram_tensor("w1", [IN_D, HID_D], F32, kind="ExternalInput")
    w2 = nc.dram_tensor("w2", [HID_D, OUT_D], F32, kind="ExternalInput")
    b1r = nc.dram_tensor("b1r", [128, D1], F32, kind="ExternalInput")
    b2r = nc.dram_tensor("b2r", [128, D2], F32, kind="ExternalInput")
    jrep = nc.dram_tensor("jrep", [128, CMAX * 128], F32, kind="ExternalInput")
    ident = nc.dram_tensor("ident", [128, 128], F32, kind="ExternalInput")
    gidx = nc.dram_tensor("gidx", [128, T_agg * 8], I16, kind="ExternalInput")
    dstl = nc.dram_tensor("dstl", [128, T_agg], I16, kind="ExternalInput")
    gidx2 = nc.dram_tensor("gidx2", [128, T_ag2 * 8], I16, kind="ExternalInput")
    dstl2 = nc.dram_tensor("dstl2", [128, T_ag2], I16, kind="ExternalInput")
    srcl = nc.dram_tensor("srcl", [128, T_deg], I16, kind="ExternalInput")

    out_loc = nc.dram_tensor("out_loc", [PB * 128, D2], F32, kind="ExternalOutput")

    xw1_lo = nc.dram_tensor("xw1_lo", [HALF, D1], F32)
    xw1_hi = nc.dram_tensor("xw1_hi", [HALF, D1], F32)
    y2w_loc_a = nc.dram_tensor("y2w_loc_a", [SPLIT * 128, D2], F32)
    y2w_loc_b = nc.dram_tensor("y2w_loc_b", [(PB - SPLIT) * 128, D2], F32)
    y2w_full_a = nc.dram_tensor("y2w_full_a", [NCORES * SPLIT * 128, D2], F32,
                                addr_space="Shared")
    y2w_full_b = nc.dram_tensor("y2w_full_b", [NCORES * (PB - SPLIT) * 128, D2], F32,
                                addr_space="Shared")
    snorm_loc = nc.dram_tensor("snorm_loc", [128, PB], F32)
    snorm_full = nc.dram_tensor("snorm_full", [NCORES * 128, PB], F32,
                                addr_space="Shared")

    rg = [list(range(NCORES))]

    with tile.TileContext(nc) as tc:
        with (
            tc.tile_pool(name="persist", bufs=1) as pp,
            tc.tile_pool(name="sbuf", bufs=2) as sb,
            tc.tile_pool(name="post", bufs=2) as pq,
            tc.tile_pool(name="psA", bufs=4, space="PSUM") as psA,
            tc.tile_pool(name="psB", bufs=2, space="PSUM") as psB,
            tc.tile_pool(name="psC", bufs=1, space="PSUM") as psC,
        ):
            # ---- constants / persistent state
            jr_t = pp.tile([128, CMAX * 128], F32)
            nc.sync.dma_start(out=jr_t[:], in_=jrep[:])
            id_t = pp.tile([128, 128], F32)
            nc.sync.dma_start(out=id_t[:], in_=ident[:])
            w1_t = pp.tile([IN_D, HID_D], F32)
            nc.sync.dma_start(out=w1_t[:], in_=w1[:])
            w2_t = pp.tile([HID_D, OUT_D], F32)
            nc.sync.dma_start(out=w2_t[:], in_=w2[:])
            b1_t = pp.tile([128, D1], F32)
            nc.sync.dma_start(out=b1_t[:], in_=b1r[:])
            b2_t = pp.tile([128, D2], F32)
            nc.sync.dma_start(out=b2_t[:], in_=b2r[:])
            ones_t = pp.tile([128, 1], F32)
            nc.vector.memset(ones_t[:], 1.0)
            gidx_t = pp.tile([128, T_agg * 8], I16)
            nc.sync.dma_start(out=gidx_t[:], in_=gidx[:])
            dstl_t = pp.tile([128, T_agg], I16)
            nc.sync.dma_start(out=dstl_t[:], in_=dstl[:])
            srcl_t = pp.tile([128, T_deg], I16)
            nc.sync.dma_start(out=srcl_t[:], in_=srcl[:])
            dstl_f = pp.tile([128, T_agg], F32)
            nc.vector.tensor_copy(dstl_f[:], dstl_t[:])
            gidx2_t = pp.tile([128, T_ag2 * 8], I16)
            nc.sync.dma_start(out=gidx2_t[:], in_=gidx2[:])
            dstl2_t = pp.tile([128, T_ag2], I16)
            nc.sync.dma_start(out=dstl2_t[:], in_=dstl2[:])
            dstl2_f = pp.tile([128, T_ag2], F32)
            nc.vector.tensor_copy(dstl2_f[:], dstl2_t[:])
            srcl_f = pp.tile([128, T_deg], F32)
            nc.vector.tensor_copy(srcl_f[:], srcl_t[:])
            s_loc = pp.tile([128, PB], F32)    # out-deg norm, own nodes
            d_loc = pp.tile([128, PB], F32)    # in-deg norm, own nodes
            s_all = pp.tile([128, NB], F32)    # out-deg norm, all nodes

            # ---- pass 1: out-degree -> s_loc
            off = 0
            for b in range(PB):
                S = Sd[b]
                deg_ps = psB.tile([128, 1], F32, space="PSUM", tag="deg")
                oh = sb.tile([128, CMAX * 128], F32, tag="ohb")
                nc.vector.tensor_tensor(
                    out=oh[:, :S * 128],
                    in0=srcl_f[:, off:off + S].to_broadcast([128, S, 128]),
                    in1=jr_t[:, :S * 128], op=mybir.AluOpType.is_equal)
                for s in range(S):
                    nc.tensor.matmul(deg_ps[:], lhsT=oh[:, s * 128:(s + 1) * 128],
                                     rhs=ones_t[:],
                                     start=(s == 0), stop=(s == S - 1))
                off += S
                t0 = pq.tile([128, 1], F32, tag="dtmp")
                nc.vector.tensor_scalar_max(t0[:], deg_ps[:], 1.0)
                t1 = pq.tile([128, 1], F32, tag="dtmp2")
                nc.scalar.activation(t1[:], t0[:], mybir.ActivationFunctionType.Sqrt)
                nc.vector.reciprocal(s_loc[:, b:b + 1], t1[:])
            nc.sync.dma_start(out=snorm_loc[:], in_=s_loc[:])
            if collectives:
                nc.gpsimd.collective_compute(
                    "AllGather", mybir.AluOpType.bypass, replica_groups=rg,
                    ins=[snorm_loc[:]], outs=[snorm_full[:]])
            else:
                for c in range(NCORES):
                    nc.sync.dma_start(out=snorm_full[c * 128:(c + 1) * 128, :],
                                      in_=snorm_loc[:])
            for c in range(NCORES):
                nc.sync.dma_start(out=s_all[:, c * PB:(c + 1) * PB],
                                  in_=snorm_full[c * 128:(c + 1) * 128, :])

            # ---- pass 2: xw1 = (x @ W1) * s  for ALL nodes (redundant per core)
            # loads batched over 8 blocks, stores over 4 (fewer DMA setups)
            GL, GS = 8, 4
            lhs = None
            t1_sb = None
            for g in range(NB if upto != 'deg' else 0):
                if g % GL == 0:
                    lhs = sb.tile([IN_D, B * GL * 128], F32, tag="t1lhs")
                    for bb in range(B):
                        nc.sync.dma_start(
                            out=lhs[:, bb * GL * 128:(bb + 1) * GL * 128],
                            in_=hT[bb, :, g * 128:(g + GL) * 128])
                if g % GS == 0:
                    t1_sb = sb.tile([128, GS * D1], F32, tag="t1sb")
                gg = g % GL
                t1_ps = psA.tile([128, D1], F32, space="PSUM", tag="bigps")
                for bb in range(B):
                    nc.tensor.matmul(
                        t1_ps[:, bb * HID_D:(bb + 1) * HID_D],
                        lhsT=lhs[:, bb * GL * 128 + gg * 128:bb * GL * 128 + (gg + 1) * 128],
                        rhs=w1_t[:], start=True, stop=True)
                nc.vector.tensor_scalar_mul(
                    t1_sb[:, (g % GS) * D1:(g % GS + 1) * D1], t1_ps[:],
                    s_all[:, g:g + 1])
                if g % GS == GS - 1:
                    g0 = g - (GS - 1)
                    tgt = xw1_lo if g0 < NB // 2 else xw1_hi
                    r0 = (g0 % (NB // 2)) * 128
                    nc.sync.dma_start(
                        out=tgt[r0:r0 + GS * 128, :].rearrange(
                            "(c p) f -> p c f", p=128),
                        in_=t1_sb[:])

            # ---- pass 3: layer-1 aggregation + layer-2 table build
            qctr = [0]

            def agg_block(b, off_sub, table_lo, table_hi, D,
                          Cls, Chs, gi_t, dl_f):
                """Emit gathers + one-hot matmuls for block b. Returns
                (agg_ps, deg_ps, n_sub)."""
                Cl, Ch = Cls[b], Chs[b]
                Ct = Cl + Ch
                g_t = sb.tile([128, Ct, D], F32, tag=f"gath{D}")
                for h, C, tab in ((0, Cl, table_lo), (1, Ch, table_hi)):
                    if C == 0:
                        continue
                    c0 = 0 if h == 0 else Cl
                    nc.gpsimd.dma_gather(
                        out_ap=g_t[:, c0:c0 + C, :], in_ap=tab[:],
                        idxs_ap=gi_t[:, (off_sub + c0) * 8:(off_sub + c0 + C) * 8],
                        num_idxs=C * 128, num_idxs_reg=C * 128,
                        elem_size=D, single_packet=False)
                agg_ps = psA.tile([128, D1], F32, space="PSUM", tag="bigps")
                if D == D1:
                    deg_ps = psB.tile([128, 1], F32, space="PSUM", tag="deg")
                else:
                    deg_ps = None
                oh = sb.tile([128, CMAX * 128], F32, tag="ohb")
                nc.vector.tensor_tensor(
                    out=oh[:, :Ct * 128],
                    in0=dl_f[:, off_sub:off_sub + Ct].to_broadcast([128, Ct, 128]),
                    in1=jr_t[:, :Ct * 128], op=mybir.AluOpType.is_equal)
                for cs in range(Ct):
                    ohc = oh[:, cs * 128:(cs + 1) * 128]
                    nc.tensor.matmul(agg_ps[:, :D], lhsT=ohc, rhs=g_t[:, cs, :],
                                     start=(cs == 0), stop=(cs == Ct - 1))
                    if D == D1:  # in-degree only needed once (layer 1)
                        nc.tensor.matmul(deg_ps[:], lhsT=ohc, rhs=ones_t[:],
                                         start=(cs == 0), stop=(cs == Ct - 1))
                return agg_ps, deg_ps, Ct

            off = 0
            for b in range(PB if upto not in ('deg', 't1') else 0):
                agg_ps, deg_ps, Ct = agg_block(b, off, xw1_lo, xw1_hi, D1,
                                               C_lo, C_hi, gidx_t, dstl_f)
                off += Ct
                # d_norm from in-degree
                t0 = pq.tile([128, 1], F32, tag="dtmp")
                nc.vector.tensor_scalar_max(t0[:], deg_ps[:], 1.0)
                t1 = pq.tile([128, 1], F32, tag="dtmp2")
                nc.scalar.activation(t1[:], t0[:], mybir.ActivationFunctionType.Sqrt)
                nc.vector.reciprocal(d_loc[:, b:b + 1], t1[:])
                # y1 = relu(agg * d + b1); y1s = y1 * s
                y1a = pq.tile([128, D1], F32, tag="y1a")
                nc.vector.tensor_scalar_mul(y1a[:], agg_ps[:], d_loc[:, b:b + 1])
                y1b = pq.tile([128, D1], F32, tag="y1b")
                nc.vector.tensor_tensor(out=y1b[:], in0=y1a[:], in1=b1_t[:],
                                        op=mybir.AluOpType.add)
                y1r = pq.tile([128, D1], F32, tag="y1r")
                nc.scalar.activation(y1r[:], y1b[:], mybir.ActivationFunctionType.Relu)
                y1s = pq.tile([128, D1], F32, tag="y1s")
                nc.vector.tensor_scalar_mul(y1s[:], y1r[:], s_loc[:, b:b + 1])
                # transform-2: y1w2 = y1s @ W2 (per batch), via PE transpose
                t2_ps = psC.tile([128, D2], F32, space="PSUM", tag="t2ps")
                for bb in range(B):
                    tr_ps = psC.tile([HID_D, 128], F32, space="PSUM", tag="trps")
                    nc.tensor.transpose(
                        tr_ps[:], y1s[:, bb * HID_D:(bb + 1) * HID_D], id_t[:])
                    tr_sb = pq.tile([HID_D, 128], F32, tag="trsb")
                    nc.vector.tensor_copy(tr_sb[:], tr_ps[:])
                    nc.tensor.matmul(
                        t2_ps[:, bb * OUT_D:(bb + 1) * OUT_D],
                        lhsT=tr_sb[:], rhs=w2_t[:], start=True, stop=True)
                t2_sb = pq.tile([128, D2], F32, tag="t2sb")
                nc.vector.tensor_copy(t2_sb[:], t2_ps[:])
                if b < SPLIT:
                    nc.sync.dma_start(out=y2w_loc_a[b * 128:(b + 1) * 128, :],
                                      in_=t2_sb[:])
                else:
                    nc.sync.dma_start(
                        out=y2w_loc_b[(b - SPLIT) * 128:(b - SPLIT + 1) * 128, :],
                        in_=t2_sb[:])
                if b == SPLIT - 1 and upto == 'l2':
                    # first table half exchanged while the rest of L1 runs
                    if collectives:
                        nc.gpsimd.collective_compute(
                            "AllGather", mybir.AluOpType.bypass, replica_groups=rg,
                            ins=[y2w_loc_a[:]], outs=[y2w_full_a[:]])
                    else:
                        for c in range(NCORES):
                            nc.sync.dma_start(
                                out=y2w_full_a[c * SPLIT * 128:(c + 1) * SPLIT * 128, :],
                                in_=y2w_loc_a[:])

            # ---- pass 4: exchange second table half
            if upto == 'l2':
                if collectives:
                    nc.gpsimd.collective_compute(
                        "AllGather", mybir.AluOpType.bypass, replica_groups=rg,
                        ins=[y2w_loc_b[:]], outs=[y2w_full_b[:]])
                else:
                    nb128 = (PB - SPLIT) * 128
                    for c in range(NCORES):
                        nc.sync.dma_start(
                            out=y2w_full_b[c * nb128:(c + 1) * nb128, :],
                            in_=y2w_loc_b[:])

            # ---- pass 5: layer-2 aggregation -> output
            off = 0
            for b in range(PB if upto == 'l2' else 0):
                agg_ps, _, Ct = agg_block(b, off, y2w_full_a, y2w_full_b, D2,
                                          C_a, C_b, gidx2_t, dstl2_f)
                off += Ct
                oa = pq.tile([128, D2], F32, tag="oa")
                nc.vector.tensor_scalar_mul(oa[:], agg_ps[:, :D2], d_loc[:, b:b + 1])
                ob = pq.tile([128, D2], F32, tag="ob")
                nc.vector.tensor_tensor(out=ob[:], in0=oa[:], in1=b2_t[:],
                                        op=mybir.AluOpType.add)
                nc.sync.dma_start(out=out_loc[b * 128:(b + 1) * 128, :], in_=ob[:])

    nc.compile()
    return nc


# ------------------------------------------------------------------- driver

def _prepare_inputs(h, W1, b1, W2, b2, src, dst):
    percore, meta = _preprocess(src, dst)
    hT = np.zeros((B, IN_D, NPAD), np.float32)
    hT[:, :, :N] = np.asarray(h, np.float32).transpose(0, 2, 1)
    b1r = np.tile(np.asarray(b1, np.float32), (128, B))
    b2r = np.tile(np.asarray(b2, np.float32), (128, B))
    cmax = max(max(meta["C_lo"][b] + meta["C_hi"][b] for b in range(PB)),
               max(meta["C_a"][b] + meta["C_b"][b] for b in range(PB)),
               max(meta["Sd"]))
    jr = np.tile(np.arange(128, dtype=np.float32), (128, cmax))
    idm = np.eye(128, dtype=np.float32)
    common = {
        "hT": hT, "w1": np.asarray(W1, np.float32), "w2": np.asarray(W2, np.float32),
        "b1r": b1r, "b2r": b2r, "jrep": jr, "ident": idm,
    }
    in_maps = [dict(common, **percore[c]) for c in range(NCORES)]
    return in_maps, meta


_BUILD_CACHE = {}


def _get_nc(meta):
    key = tuple(sorted((k, tuple(v) if isinstance(v, list) else v)
                       for k, v in meta.items()))
    if key not in _BUILD_CACHE:
        nc = _build(meta)
        nc.m = get_hw_module(nc.m)
        _BUILD_CACHE[key] = nc
    return _BUILD_CACHE[key]


def _assemble(results):
    full = np.concatenate([results[c]["out_loc"] for c in range(NCORES)], axis=0)
    out = full.reshape(NPAD, B, OUT_D).transpose(1, 0, 2)[:, :N, :]
    return np.ascontiguousarray(out, dtype=np.float32)


def kernel(h, W1, b1, W2, b2, src, dst):
    in_maps, meta = _prepare_inputs(h, W1, b1, W2, b2, src, dst)
    nc = _get_nc(meta)
    res = run_bass_kernel_spmd(nc, in_maps, core_ids=list(range(NCORES)))
    return _assemble(res.results)



# revision 5
# speedup vs baseline: 2.0762x; 2.0762x over previous
"""Trainium2 Bass kernel for a 2-layer GraphConv GCN (nn_GCNN_69776038691375).

reference semantics:
    x = h.swapaxes(0,1)                       # [N, B, F]
    out_deg/in_deg from src/dst, clipped at 1
    s = out_deg**-0.5 ; d = in_deg**-0.5
    layer(x, W, b) = (segsum((x*s)[src] -> dst) * d) @ W + b
    y = relu(layer(x, W1, b1)); out = layer(y, W2, b2); return out.swapaxes(0,1)

Design (v2):
  * Degree norms are topology-only -> computed on host (bincount), shipped as
    tiny per-node scale vectors. No on-device degree pass.
  * Layer-1 gathers read rows of hB = (x*s) directly (host-prescaled, bf16,
    512B rows) -- no on-device x@W1 pre-pass. W1 is applied AFTER aggregation
    (agg @ W1 via PE transposes), which is cheap because it runs once per
    dst block instead of once per node on every core.
  * Layer-2 gathers read rows of y2w = (relu(d*agg@W1+b1)*s) @ W2 (bf16,
    256B rows), exchanged via two AllGathers (first fires early to overlap).
  * dst-node sharding: core c owns blocks [c*49, (c+1)*49) of 128 nodes.
    Edges grouped per dst block, split lo/hi by src < 25088 (int16 gather
    indices). Aggregation = dma_gather + one-hot (is_equal vs iota) matmuls
    accumulating in PSUM, all bf16 (PE 1 cycle/row).
  * Gathers are chunked over G blocks per call to amortize the SWDGE
    fixed descriptor-generation overhead on the Pool engine.
"""

import numpy as np
import ml_dtypes

import concourse.bacc as bacc
import concourse.bass as bass
import concourse.mybir as mybir
import concourse.tile as tile
from concourse.bass_interp import get_hw_module
from concourse.bass_utils import run_bass_kernel_spmd

F32 = mybir.dt.float32
BF16 = mybir.dt.bfloat16
I16 = mybir.dt.int16
NPBF16 = ml_dtypes.bfloat16

# problem sizes (hardcoded per contract)
N = 50000
E = 800000
B = 4
IN_D, HID_D, OUT_D = 64, 64, 32
NCORES = 8
PB = 49                 # blocks per core
NB = NCORES * PB        # 392 global blocks
NPAD = NB * 128         # 50176
HALF = NPAD // 2        # 25088: dma_gather int16 index limit split point
D1 = B * HID_D          # 256 bf16 per hB row (512B)
D2 = B * OUT_D          # 128 bf16 per y2w row (256B)
SENT = 250              # one-hot sentinel for padded edges
SPLIT = 24              # L1 block index after which the first y2w AllGather fires
G1 = 7                  # L1 blocks per gather chunk
G2 = 13                 # L2 blocks per gather chunk


def _chunks(g):
    return [list(range(i, min(i + g, PB))) for i in range(0, PB, g)]


# ---------------------------------------------------------------- host side

def _wrap_idx(flat):
    """dma_gather index layout: idx j of a gather lives at [j%16, j//16],
    replicated across the 8 groups of 16 partitions. flat: [T, 128] int16
    (subtile-major). Returns [128, T*8]."""
    T = flat.shape[0]
    w = flat.reshape(T, 8, 16).transpose(2, 0, 1).reshape(16, T * 8)
    return np.tile(w, (8, 1)).astype(np.int16)


def _edge_struct(src_s, dst_s, key_s, cnt, starts, Cs0, Cs1, idx_of):
    """Build per-core (gidx chunk-ordered, dstl block-ordered) int16 arrays.

    src_s/dst_s/key_s: globally sorted edge arrays (by (block, key, src)).
    cnt/starts: [NB, 2] per (global block, key half).
    Cs0/Cs1: per block-index subtile counts (len PB), max over cores.
    idx_of: fn(src_values, half) -> int16 gather indices.
    """
    percore = []
    chunks = None
    for c in range(NCORES):
        sub_g = {}
        sub_d = {}
        for b in range(PB):
            g = c * PB + b
            base = g * 128
            for h, C in ((0, Cs0[b]), (1, Cs1[b])):
                n = int(cnt[g, h])
                st = int(starts[g, h])
                gi = np.zeros(C * 128, np.int16)
                dl = np.full(C * 128, SENT, np.int16)
                gi[:n] = idx_of(src_s[st:st + n], h)
                dl[:n] = (dst_s[st:st + n] - base).astype(np.int16)
                sub_g[(b, h)] = gi.reshape(C, 128)
                sub_d[(b, h)] = dl.reshape(C, 128)
        percore.append((sub_g, sub_d))
    return percore


def _pack(percore_sub, Cs0, Cs1, chunks):
    """gidx: chunk-ordered (per chunk: lo of all blocks, then hi);
    dstl: block-ordered (per block: lo subtiles then hi)."""
    out = []
    for sub_g, sub_d in percore_sub:
        gs = []
        for ch in chunks:
            for b in ch:
                gs.append(sub_g[(b, 0)])
            for b in ch:
                gs.append(sub_g[(b, 1)])
        ds = []
        for b in range(PB):
            ds.append(sub_d[(b, 0)])
            ds.append(sub_d[(b, 1)])
        gidx = _wrap_idx(np.concatenate(gs, axis=0))
        dstl = np.ascontiguousarray(np.concatenate(ds, axis=0).T)
        out.append((gidx, dstl.astype(NPBF16)))
    return out


def _preprocess(src, dst):
    src = np.asarray(src).astype(np.int64)
    dst = np.asarray(dst).astype(np.int64)

    # degree norms (topology only -> host)
    s_norm = np.maximum(np.bincount(src, minlength=N), 1.0) ** -0.5
    d_norm = np.maximum(np.bincount(dst, minlength=N), 1.0) ** -0.5
    s_pad = np.ones(NPAD, np.float64)
    d_pad = np.ones(NPAD, np.float64)
    s_pad[:N] = s_norm
    d_pad[:N] = d_norm

    blk = dst >> 7

    # ---- L1: dst-sorted, split by src half (gather from hb_lo / hb_hi)
    hi = (src >= HALF).astype(np.int64)
    order = np.lexsort((src, hi, blk))
    s_src, s_dst = src[order], dst[order]
    cnt = np.bincount(blk[order] * 2 + hi[order], minlength=NB * 2).reshape(NB, 2)
    starts = np.concatenate([[0], np.cumsum(cnt.ravel())])[:-1].reshape(NB, 2)
    lo_sub = -(-cnt[:, 0] // 128).reshape(NCORES, PB)
    hi_sub = -(-cnt[:, 1] // 128).reshape(NCORES, PB)
    C_lo = np.maximum(lo_sub.max(axis=0), 1).astype(int)
    C_hi = hi_sub.max(axis=0).astype(int)
    pc1 = _edge_struct(s_src, s_dst, None, cnt, starts, C_lo, C_hi,
                       lambda s, h: (s - h * HALF).astype(np.int16))

    # ---- L2: dst-sorted, split by whether src block-index >= SPLIT
    # (gather from the two AllGather output tables A / B)
    src_c = src // (PB * 128)
    src_b = (src % (PB * 128)) >> 7
    src_p = src & 127
    in_b2 = (src_b >= SPLIT).astype(np.int64)
    pos = np.where(in_b2 == 0,
                   src_c * (SPLIT * 128) + src_b * 128 + src_p,
                   src_c * ((PB - SPLIT) * 128) + (src_b - SPLIT) * 128 + src_p)
    order2 = np.lexsort((src, in_b2, blk))
    t_pos, t_dst = pos[order2], dst[order2]
    cnt2 = np.bincount(blk[order2] * 2 + in_b2[order2],
                       minlength=NB * 2).reshape(NB, 2)
    starts2 = np.concatenate([[0], np.cumsum(cnt2.ravel())])[:-1].reshape(NB, 2)
    a_sub = -(-cnt2[:, 0] // 128).reshape(NCORES, PB)
    b_sub = -(-cnt2[:, 1] // 128).reshape(NCORES, PB)
    C_a = np.maximum(a_sub.max(axis=0), 1).astype(int)
    C_b = b_sub.max(axis=0).astype(int)
    pc2 = _edge_struct(t_pos, t_dst, None, cnt2, starts2, C_a, C_b,
                       lambda s, h: s.astype(np.int16))

    packed1 = _pack(pc1, C_lo, C_hi, _chunks(G1))
    packed2 = _pack(pc2, C_a, C_b, _chunks(G2))
    percore = [{"gidx": packed1[c][0], "dstl": packed1[c][1],
                "gidx2": packed2[c][0], "dstl2": packed2[c][1]}
               for c in range(NCORES)]
    meta = dict(C_lo=C_lo.tolist(), C_hi=C_hi.tolist(),
                C_a=C_a.tolist(), C_b=C_b.tolist())
    return percore, meta, s_pad, d_pad


# -------------------------------------------------------------- bass program

def _build(meta, collectives=True, upto='l2'):
    C_lo, C_hi = meta["C_lo"], meta["C_hi"]
    C_a, C_b = meta["C_a"], meta["C_b"]
    b1z, b2z = meta["b1z"], meta["b2z"]
    T1 = sum(C_lo) + sum(C_hi)
    T2 = sum(C_a) + sum(C_b)
    CMAX = max(max(C_lo[b] + C_hi[b] for b in range(PB)),
               max(C_a[b] + C_b[b] for b in range(PB)))
    ch1, ch2 = _chunks(G1), _chunks(G2)
    SLOT1 = max(sum(C_lo[b] + C_hi[b] for b in ch) for ch in ch1)
    SLOT2 = max(sum(C_a[b] + C_b[b] for b in ch) for ch in ch2)

    nc = bacc.Bacc("TRN2", target_bir_lowering=False, debug=False,
                   num_devices=NCORES)

    hb_lo = nc.dram_tensor("hb_lo", [HALF, D1], BF16, kind="ExternalInput")
    hb_hi = nc.dram_tensor("hb_hi", [HALF, D1], BF16, kind="ExternalInput")
    w1 = nc.dram_tensor("w1", [IN_D, HID_D], BF16, kind="ExternalInput")
    w2 = nc.dram_tensor("w2", [HID_D, OUT_D], BF16, kind="ExternalInput")
    dn = nc.dram_tensor("dn", [128, PB], F32, kind="ExternalInput")
    sdn = nc.dram_tensor("sdn", [128, PB], F32, kind="ExternalInput")
    b1r = nc.dram_tensor("b1r", [128, D1], F32, kind="ExternalInput")
    b2r = nc.dram_tensor("b2r", [128, D2], F32, kind="ExternalInput")
    jrep = nc.dram_tensor("jrep", [128, CMAX * 128], BF16, kind="ExternalInput")
    ident = nc.dram_tensor("ident", [128, 128], BF16, kind="ExternalInput")
    gidx = nc.dram_tensor("gidx", [128, T1 * 8], I16, kind="ExternalInput")
    dstl = nc.dram_tensor("dstl", [128, T1], BF16, kind="ExternalInput")
    gidx2 = nc.dram_tensor("gidx2", [128, T2 * 8], I16, kind="ExternalInput")
    dstl2 = nc.dram_tensor("dstl2", [128, T2], BF16, kind="ExternalInput")

    out_loc = nc.dram_tensor("out_loc", [PB * 128, D2], F32, kind="ExternalOutput")

    y2w_loc_a = nc.dram_tensor("y2w_loc_a", [SPLIT * 128, D2], BF16)
    y2w_loc_b = nc.dram_tensor("y2w_loc_b", [(PB - SPLIT) * 128, D2], BF16)
    y2w_full_a = nc.dram_tensor("y2w_full_a", [NCORES * SPLIT * 128, D2], BF16,
                                addr_space="Shared")
    y2w_full_b = nc.dram_tensor("y2w_full_b", [NCORES * (PB - SPLIT) * 128, D2],
                                BF16, addr_space="Shared")

    rg = [list(range(NCORES))]
    EQ = mybir.AluOpType.is_equal
    RELU = mybir.ActivationFunctionType.Relu
    COPY = mybir.ActivationFunctionType.Copy

    with tile.TileContext(nc) as tc:
        with (
            tc.tile_pool(name="persist", bufs=1) as pp,
            tc.tile_pool(name="sbuf", bufs=2) as sb,
            tc.tile_pool(name="post", bufs=2) as pq,
            tc.tile_pool(name="psA", bufs=2, space="PSUM") as psA,
            tc.tile_pool(name="psW", bufs=2, space="PSUM") as psW,
            tc.tile_pool(name="psT", bufs=2, space="PSUM") as psT,
            tc.tile_pool(name="psY", bufs=2, space="PSUM") as psY,
        ):
            # ---- persistent constants
            jr_t = pp.tile([128, CMAX * 128], BF16)
            nc.sync.dma_start(out=jr_t[:], in_=jrep[:])
            id_t = pp.tile([128, 128], BF16)
            nc.sync.dma_start(out=id_t[:], in_=ident[:])
            w1_t = pp.tile([IN_D, HID_D], BF16)
            nc.sync.dma_start(out=w1_t[:], in_=w1[:])
            w2_t = pp.tile([HID_D, OUT_D], BF16)
            nc.sync.dma_start(out=w2_t[:], in_=w2[:])
            d_t = pp.tile([128, PB], F32)
            nc.sync.dma_start(out=d_t[:], in_=dn[:])
            sd_t = pp.tile([128, PB], F32)
            nc.sync.dma_start(out=sd_t[:], in_=sdn[:])
            dstl_t = pp.tile([128, T1], BF16)
            nc.sync.dma_start(out=dstl_t[:], in_=dstl[:])
            dstl2_t = pp.tile([128, T2], BF16)
            nc.sync.dma_start(out=dstl2_t[:], in_=dstl2[:])
            if not b1z:
                b1_t = pp.tile([128, D1], F32)
                nc.sync.dma_start(out=b1_t[:], in_=b1r[:])
            if not b2z:
                b2_t = pp.tile([128, D2], F32)
                nc.sync.dma_start(out=b2_t[:], in_=b2r[:])

            # ---- layer 1: chunked gathers from hB, aggregate, transform
            goff = 0   # subtile offset into gidx (chunk order)
            doff = 0   # subtile offset into dstl (block order)
            for ch in (ch1 if upto != 'none' else []):
                CL = sum(C_lo[b] for b in ch)
                CH = sum(C_hi[b] for b in ch)
                CT = CL + CH
                gx = sb.tile([128, max(SLOT1, SLOT2) * 8], I16, tag="gx")
                nc.sync.dma_start(out=gx[:, :CT * 8],
                                  in_=gidx[:, goff * 8:(goff + CT) * 8])
                gt = sb.tile([128, SLOT1, D1], BF16, tag="gath")
                if CL:
                    nc.gpsimd.dma_gather(
                        out_ap=gt[:, :CL, :], in_ap=hb_lo[:],
                        idxs_ap=gx[:, :CL * 8],
                        num_idxs=CL * 128, num_idxs_reg=CL * 128,
                        elem_size=D1, single_packet=False)
                if CH:
                    nc.gpsimd.dma_gather(
                        out_ap=gt[:, CL:CT, :], in_ap=hb_hi[:],
                        idxs_ap=gx[:, CL * 8:CT * 8],
                        num_idxs=CH * 128, num_idxs_reg=CH * 128,
                        elem_size=D1, single_packet=False)
                lo_base = 0
                hi_base = CL
                for b in ch:
                    Cl, Chh = C_lo[b], C_hi[b]
                    Ct = Cl + Chh
                    oh = sb.tile([128, CMAX * 128], BF16, tag="oh")
                    nc.vector.tensor_tensor(
                        out=oh[:, :Ct * 128],
                        in0=dstl_t[:, doff:doff + Ct].to_broadcast([128, Ct, 128]),
                        in1=jr_t[:, :Ct * 128], op=EQ)
                    agg_ps = psA.tile([128, D1], F32, space="PSUM", tag="agg")
                    for k in range(Ct):
                        rhs = (gt[:, lo_base + k, :] if k < Cl
                               else gt[:, hi_base + (k - Cl), :])
                        nc.tensor.matmul(agg_ps[:], lhsT=oh[:, k * 128:(k + 1) * 128],
                                         rhs=rhs, start=(k == 0), stop=(k == Ct - 1))
                    lo_base += Cl
                    hi_base += Chh
                    doff += Ct
                    # z = agg @ W1   (via PE transposes, per batch)
                    agg_sb = pq.tile([128, D1], BF16, tag="aggsb")
                    nc.scalar.copy(agg_sb[:], agg_ps[:])
                    zW_ps = psW.tile([128, D1], F32, space="PSUM", tag="zw")
                    for bb in range(B):
                        tr_ps = psT.tile([HID_D, 128], BF16, space="PSUM", tag="tr")
                        nc.tensor.transpose(
                            tr_ps[:], agg_sb[:, bb * HID_D:(bb + 1) * HID_D], id_t[:])
                        tr_sb = pq.tile([HID_D, 128], BF16, tag="trsb")
                        nc.scalar.copy(tr_sb[:], tr_ps[:])
                        nc.tensor.matmul(
                            zW_ps[:, bb * HID_D:(bb + 1) * HID_D],
                            lhsT=tr_sb[:], rhs=w1_t[:], start=True, stop=True)
                    # y1 = relu(d*z + b1); table row = (y1*s*d... ) see sd scale
                    y1r = pq.tile([128, D1], BF16, tag="y1r")
                    if b1z:
                        nc.scalar.activation(y1r[:], zW_ps[:], RELU,
                                             scale=d_t[:, b:b + 1])
                    else:
                        t0 = pq.tile([128, D1], F32, tag="zb0")
                        nc.vector.tensor_scalar_mul(t0[:], zW_ps[:], d_t[:, b:b + 1])
                        t1 = pq.tile([128, D1], F32, tag="zb1")
                        nc.vector.tensor_tensor(out=t1[:], in0=t0[:], in1=b1_t[:],
                                                op=mybir.AluOpType.add)
                        nc.scalar.activation(y1r[:], t1[:], RELU)
                    # y2w row = (y1 * s) @ W2; s folded in via sd post-scale:
                    # (relu-part) @ W2 scaled per node by s (d already applied)
                    y2w_ps = psY.tile([128, D2], F32, space="PSUM", tag="y2w")
                    for bb in range(B):
                        tr2_ps = psT.tile([HID_D, 128], BF16, space="PSUM", tag="tr")
                        nc.tensor.transpose(
                            tr2_ps[:], y1r[:, bb * HID_D:(bb + 1) * HID_D], id_t[:])
                        tr2_sb = pq.tile([HID_D, 128], BF16, tag="trsb")
                        nc.scalar.copy(tr2_sb[:], tr2_ps[:])
                        nc.tensor.matmul(
                            y2w_ps[:, bb * OUT_D:(bb + 1) * OUT_D],
                            lhsT=tr2_sb[:], rhs=w2_t[:], start=True, stop=True)
                    y2w_sb = pq.tile([128, D2], BF16, tag="y2wsb")
                    # s-scale for the next layer's gather (d already in y1r)
                    nc.scalar.activation(y2w_sb[:], y2w_ps[:], COPY,
                                         scale=sd_t[:, b:b + 1])
                    if b < SPLIT:
                        nc.sync.dma_start(
                            out=y2w_loc_a[b * 128:(b + 1) * 128, :], in_=y2w_sb[:])
                    else:
                        nc.sync.dma_start(
                            out=y2w_loc_b[(b - SPLIT) * 128:(b - SPLIT + 1) * 128, :],
                            in_=y2w_sb[:])
                    if b == SPLIT - 1 and upto == 'l2':
                        if collectives:
                            nc.gpsimd.collective_compute(
                                "AllGather", mybir.AluOpType.bypass,
                                replica_groups=rg,
                                ins=[y2w_loc_a[:]], outs=[y2w_full_a[:]])
                        else:
                            for c in range(NCORES):
                                nc.sync.dma_start(
                                    out=y2w_full_a[c * SPLIT * 128:(c + 1) * SPLIT * 128, :],
                                    in_=y2w_loc_a[:])
                goff += CT

            # ---- exchange second table half
            if upto == 'l2':
                if collectives:
                    nc.gpsimd.collective_compute(
                        "AllGather", mybir.AluOpType.bypass, replica_groups=rg,
                        ins=[y2w_loc_b[:]], outs=[y2w_full_b[:]])
                else:
                    nb128 = (PB - SPLIT) * 128
                    for c in range(NCORES):
                        nc.sync.dma_start(
                            out=y2w_full_b[c * nb128:(c + 1) * nb128, :],
                            in_=y2w_loc_b[:])

            # ---- layer 2
            goff = 0
            doff = 0
            for ch in (ch2 if upto == 'l2' else []):
                CA = sum(C_a[b] for b in ch)
                CB = sum(C_b[b] for b in ch)
                CT = CA + CB
                gx = sb.tile([128, max(SLOT1, SLOT2) * 8], I16, tag="gx")
                nc.sync.dma_start(out=gx[:, :CT * 8],
                                  in_=gidx2[:, goff * 8:(goff + CT) * 8])
                g2v = sb.tile([128, SLOT2, D2], BF16, tag="gath")
                if CA:
                    nc.gpsimd.dma_gather(
                        out_ap=g2v[:, :CA, :], in_ap=y2w_full_a[:],
                        idxs_ap=gx[:, :CA * 8],
                        num_idxs=CA * 128, num_idxs_reg=CA * 128,
                        elem_size=D2, single_packet=False)
                if CB:
                    nc.gpsimd.dma_gather(
                        out_ap=g2v[:, CA:CT, :], in_ap=y2w_full_b[:],
                        idxs_ap=gx[:, CA * 8:CT * 8],
                        num_idxs=CB * 128, num_idxs_reg=CB * 128,
                        elem_size=D2, single_packet=False)
                a_base = 0
                b_base = CA
                for b in ch:
                    Ca, Cb = C_a[b], C_b[b]
                    Ct = Ca + Cb
                    oh = sb.tile([128, CMAX * 128], BF16, tag="oh")
                    nc.vector.tensor_tensor(
                        out=oh[:, :Ct * 128],
                        in0=dstl2_t[:, doff:doff + Ct].to_broadcast([128, Ct, 128]),
                        in1=jr_t[:, :Ct * 128], op=EQ)
                    agg_ps = psA.tile([128, D1], F32, space="PSUM", tag="agg")
                    for k in range(Ct):
                        rhs = (g2v[:, a_base + k, :] if k < Ca
                               else g2v[:, b_base + (k - Ca), :])
                        nc.tensor.matmul(agg_ps[:, :D2],
                                         lhsT=oh[:, k * 128:(k + 1) * 128],
                                         rhs=rhs, start=(k == 0), stop=(k == Ct - 1))
                    a_base += Ca
                    b_base += Cb
                    doff += Ct
                    out_sb = pq.tile([128, D2], F32, tag="outsb")
                    if b2z:
                        nc.scalar.activation(out_sb[:], agg_ps[:, :D2], COPY,
                                             scale=d_t[:, b:b + 1])
                    else:
                        t0 = pq.tile([128, D2], F32, tag="ob0")
                        nc.vector.tensor_scalar_mul(t0[:], agg_ps[:, :D2],
                                                    d_t[:, b:b + 1])
                        nc.vector.tensor_tensor(out=out_sb[:], in0=t0[:],
                                                in1=b2_t[:], op=mybir.AluOpType.add)
                    nc.sync.dma_start(out=out_loc[b * 128:(b + 1) * 128, :],
                                      in_=out_sb[:])
                goff += CT

    nc.compile()
    return nc


# ------------------------------------------------------------------- driver

def _prepare_inputs(h, W1, b1, W2, b2, src, dst):
    percore, meta, s_pad, d_pad = _preprocess(src, dst)
    meta["b1z"] = bool(np.all(np.asarray(b1) == 0))
    meta["b2z"] = bool(np.all(np.asarray(b2) == 0))

    # hB rows: node-major [n, B*F], pre-scaled by s_norm, bf16
    hs = np.asarray(h, np.float32).transpose(1, 0, 2).reshape(N, B * IN_D)
    hs = hs * s_pad[:N, None].astype(np.float32)
    hb = np.zeros((NPAD, D1), NPBF16)
    hb[:N] = hs.astype(NPBF16)

    CMAX = max(max(meta["C_lo"][b] + meta["C_hi"][b] for b in range(PB)),
               max(meta["C_a"][b] + meta["C_b"][b] for b in range(PB)))
    jr = np.tile(np.arange(128, dtype=np.float32), (128, CMAX)).astype(NPBF16)
    idm = np.eye(128, dtype=np.float32).astype(NPBF16)

    nodes = np.arange(NPAD)
    d_all = d_pad[nodes].reshape(NCORES, PB, 128)
    sd_all = s_pad[nodes].reshape(NCORES, PB, 128)

    common = {
        "hb_lo": hb[:HALF], "hb_hi": hb[HALF:],
        "w1": np.asarray(W1, np.float32).astype(NPBF16),
        "w2": np.asarray(W2, np.float32).astype(NPBF16),
        "b1r": np.tile(np.asarray(b1, np.float32), (128, B)),
        "b2r": np.tile(np.asarray(b2, np.float32), (128, B)),
        "jrep": jr, "ident": idm,
    }
    in_maps = []
    for c in range(NCORES):
        m = dict(common, **percore[c])
        m["dn"] = np.ascontiguousarray(d_all[c].T, dtype=np.float32)
        m["sdn"] = np.ascontiguousarray(sd_all[c].T, dtype=np.float32)
        in_maps.append(m)
    return in_maps, meta


_BUILD_CACHE = {}


def _get_nc(meta):
    key = tuple(sorted((k, tuple(v) if isinstance(v, list) else v)
                       for k, v in meta.items()))
    if key not in _BUILD_CACHE:
        nc = _build(meta)
        nc.m = get_hw_module(nc.m)
        _BUILD_CACHE[key] = nc
    return _BUILD_CACHE[key]


def _assemble(results):
    full = np.concatenate([results[c]["out_loc"] for c in range(NCORES)], axis=0)
    out = full.reshape(NPAD, B, OUT_D).transpose(1, 0, 2)[:, :N, :]
    return np.ascontiguousarray(out, dtype=np.float32)


def kernel(h, W1, b1, W2, b2, src, dst):
    in_maps, meta = _prepare_inputs(h, W1, b1, W2, b2, src, dst)
    nc = _get_nc(meta)
    res = run_bass_kernel_spmd(nc, in_maps, core_ids=list(range(NCORES)))
    return _assemble(res.results)


# revision 6
# speedup vs baseline: 2.1459x; 1.0336x over previous
"""Trainium2 Bass kernel for a 2-layer GraphConv GCN (nn_GCNN_69776038691375).

reference semantics:
    x = h.swapaxes(0,1)                       # [N, B, F]
    out_deg/in_deg from src/dst, clipped at 1
    s = out_deg**-0.5 ; d = in_deg**-0.5
    layer(x, W, b) = (segsum((x*s)[src] -> dst) * d) @ W + b
    y = relu(layer(x, W1, b1)); out = layer(y, W2, b2); return out.swapaxes(0,1)

Design (v3):
  * Degree norms are topology-only -> computed on host (bincount), shipped as
    tiny per-node scale vectors. No on-device degree pass.
  * Layer-1 gathers read rows of hB = (x*s) directly (host-prescaled, bf16,
    512B rows) -- W1 is applied after aggregation per dst block.
  * Layer-2 gathers rows of y2w = (y1*s) @ W2 (bf16, 256B rows), exchanged
    via two AllGathers (the first fires early to overlap with layer 1).
  * dst-node sharding: core c owns blocks [c*49, (c+1)*49) of 128 nodes.
  * Hybrid aggregation: for each dst-local slot j, its first <=M edges (per
    src-table) are placed at partition j of "identity subtiles" -> the
    aggregation matmul uses a constant identity lhsT (no one-hot build).
    Overflow edges go to packed subtiles reduced with a one-hot built by
    is_equal vs iota (DVE). Empty identity slots gather a guaranteed-zero
    row: two nodes are host-swapped with pad slots so every gather table
    has a zero row (pads also get s=0 so their y2w rows vanish).
  * Gathers are chunked over several blocks per dma_gather call to amortize
    the SWDGE fixed descriptor-generation overhead on the Pool engine.
"""

import numpy as np
import ml_dtypes

import concourse.bacc as bacc
import concourse.bass as bass
import concourse.mybir as mybir
import concourse.tile as tile
from concourse.bass_interp import get_hw_module
from concourse.bass_utils import run_bass_kernel_spmd

F32 = mybir.dt.float32
BF16 = mybir.dt.bfloat16
I16 = mybir.dt.int16
NPBF16 = ml_dtypes.bfloat16

# problem sizes (hardcoded per contract)
N = 50000
E = 800000
B = 4
IN_D, HID_D, OUT_D = 64, 64, 32
NCORES = 8
PB = 49                 # blocks per core
NB = NCORES * PB        # 392 global blocks
NPAD = NB * 128         # 50176
HALF = NPAD // 2        # 25088: dma_gather int16 index limit split point
D1 = B * HID_D          # 256 bf16 per hB row (512B)
D2 = B * OUT_D          # 128 bf16 per y2w row (256B)
SENT = 250              # one-hot sentinel for padded edges
SPLIT = 24              # L1 block index after which the first y2w AllGather fires
G1 = 7                  # L1 blocks per gather chunk
G2 = 13                 # L2 blocks per gather chunk
M = 6                   # identity-subtile depth per (block, table)

# node<->slot permutation: slots 127 and 3199 become pads (zero rows for the
# lo / A / B gather tables); their nodes move to the tail pad slots. The hi
# table's zero row is the untouched pad slot 50000.
SWAPS = ((127, NPAD - 2), (3199, NPAD - 1))
Z_LO = 127
Z_HI = 50000 - HALF
Z_A = 127               # slot 127: block 0 < SPLIT, posA = 127
Z_B = 127               # slot 3199: block 24, posB = 127


def _chunks(g):
    return [list(range(i, min(i + g, PB))) for i in range(0, PB, g)]


# ---------------------------------------------------------------- host side

def _wrap_idx(flat):
    """dma_gather index layout: idx j of a gather lives at [j%16, j//16],
    replicated across the 8 groups of 16 partitions. flat: [T, 128] int16
    (subtile-major). Returns [128, T*8]."""
    T = flat.shape[0]
    w = flat.reshape(T, 8, 16).transpose(2, 0, 1).reshape(16, T * 8)
    return np.tile(w, (8, 1)).astype(np.int16)


def _place_block(j_arr, idx_arr, zidx):
    """Distribute one (core, block, table) edge slice.

    Each dst-local j gets its first <=M edges at partition j of identity
    subtiles 0..M-1 (empty slots -> zidx, a zero row). Returns
    (id_idx [M,128] int16, left_idx, left_j) for the overflow edges."""
    order = np.argsort(j_arr, kind="stable")
    j_s = j_arr[order]
    s_s = idx_arr[order]
    n = len(j_s)
    if n:
        newgrp = np.concatenate([[True], j_s[1:] != j_s[:-1]])
        gstart = np.maximum.accumulate(np.where(newgrp, np.arange(n), 0))
        rank = np.arange(n) - gstart
    else:
        rank = np.zeros(0, np.int64)
    idm = rank < M
    id_idx = np.full((M, 128), zidx, np.int16)
    id_idx[rank[idm], j_s[idm]] = s_s[idm]
    return id_idx, s_s[~idm], j_s[~idm]


def _preprocess(src, dst):
    src = np.asarray(src).astype(np.int64)
    dst = np.asarray(dst).astype(np.int64)

    # node -> slot permutation
    slot_of = np.arange(NPAD, dtype=np.int64)
    for a, b in SWAPS:
        slot_of[a], slot_of[b] = slot_of[b], slot_of[a]
    src = slot_of[src]
    dst = slot_of[dst]

    # degree norms by slot (topology only -> host). Pads: s=0 (kills their
    # y2w rows even with nonzero bias), d=1.
    s_pad = np.zeros(NPAD, np.float64)
    d_pad = np.ones(NPAD, np.float64)
    s_cnt = np.bincount(src, minlength=NPAD).astype(np.float64)
    d_cnt = np.bincount(dst, minlength=NPAD).astype(np.float64)
    real = np.zeros(NPAD, bool)
    real[slot_of[:N]] = True
    s_pad[real] = np.maximum(s_cnt[real], 1.0) ** -0.5
    d_pad[real] = np.maximum(d_cnt[real], 1.0) ** -0.5

    blk = dst >> 7
    dloc = dst & 127

    # L1 tables: lo/hi by src slot half; L2 tables: A/B by src block < SPLIT
    t1 = (src >= HALF).astype(np.int64)
    i1 = src - t1 * HALF
    src_c = src // (PB * 128)
    src_b = (src % (PB * 128)) >> 7
    src_p = src & 127
    t2 = (src_b >= SPLIT).astype(np.int64)
    i2 = np.where(t2 == 0,
                  src_c * (SPLIT * 128) + src_b * 128 + src_p,
                  src_c * ((PB - SPLIT) * 128) + (src_b - SPLIT) * 128 + src_p)

    def build(tt, ii, z0, z1, chunks):
        order = np.lexsort((ii, tt, blk))
        o_blk, o_t, o_i, o_j = blk[order], tt[order], ii[order], dloc[order]
        cnt = np.bincount(o_blk * 2 + o_t, minlength=NB * 2).reshape(NB, 2)
        starts = np.concatenate([[0], np.cumsum(cnt.ravel())])[:-1].reshape(NB, 2)
        id_idx = {}
        left = {}
        nleft = np.zeros((NB, 2), np.int64)
        for g in range(NB):
            for t in range(2):
                st, n = int(starts[g, t]), int(cnt[g, t])
                z = z0 if t == 0 else z1
                idt, li, lj = _place_block(o_j[st:st + n], o_i[st:st + n], z)
                id_idx[(g, t)] = idt
                left[(g, t)] = (li, lj)
                nleft[g, t] = len(li)
        Lsub = (-(-nleft // 128)).reshape(NCORES, PB, 2).max(axis=0)  # [PB, 2]
        L0, L1 = Lsub[:, 0].astype(int), Lsub[:, 1].astype(int)
        percore = []
        for c in range(NCORES):
            gs = []      # chunk-ordered gather subtiles
            ds = []      # block-ordered one-hot dst-locals (leftovers only)
            for ch in chunks:
                for t in range(2):
                    for b in ch:
                        g = c * PB + b
                        L = int((L0 if t == 0 else L1)[b])
                        gs.append(id_idx[(g, t)])
                        li, lj = left[(g, t)]
                        z = z0 if t == 0 else z1
                        gi = np.full(L * 128, z, np.int16)
                        gi[:len(li)] = li.astype(np.int16)
                        gs.append(gi.reshape(L, 128))
            for b in range(PB):
                for t in range(2):
                    g = c * PB + b
                    L = int((L0 if t == 0 else L1)[b])
                    li, lj = left[(g, t)]
                    dl = np.full(L * 128, SENT, np.int16)
                    dl[:len(lj)] = lj.astype(np.int16)
                    ds.append(dl.reshape(L, 128))
            gidx = _wrap_idx(np.concatenate(gs, axis=0))
            dstl = np.ascontiguousarray(
                np.concatenate(ds, axis=0).T).astype(NPBF16)
            percore.append((gidx, dstl))
        return percore, L0.tolist(), L1.tolist()

    pc1, L_lo, L_hi = build(t1, i1, Z_LO, Z_HI, _chunks(G1))
    pc2, L_a, L_b = build(t2, i2, Z_A, Z_B, _chunks(G2))

    percore = [{"gidx": pc1[c][0], "dstl": pc1[c][1],
                "gidx2": pc2[c][0], "dstl2": pc2[c][1]}
               for c in range(NCORES)]
    meta = dict(L_lo=L_lo, L_hi=L_hi, L_a=L_a, L_b=L_b)
    return percore, meta, s_pad, d_pad, slot_of


# -------------------------------------------------------------- bass program

def _build(meta, collectives=True, upto='l2'):
    L_lo, L_hi = meta["L_lo"], meta["L_hi"]
    L_a, L_b = meta["L_a"], meta["L_b"]
    b1z, b2z = meta["b1z"], meta["b2z"]

    def ct(L0, L1, b):
        return 2 * M + L0[b] + L1[b]

    T1 = sum(ct(L_lo, L_hi, b) for b in range(PB))
    T2 = sum(ct(L_a, L_b, b) for b in range(PB))
    T1L = sum(L_lo) + sum(L_hi)
    T2L = sum(L_a) + sum(L_b)
    LMAX = max(max(L_lo[b] + L_hi[b] for b in range(PB)),
               max(L_a[b] + L_b[b] for b in range(PB)), 1)
    ch1, ch2 = _chunks(G1), _chunks(G2)
    SLOT1 = max(sum(ct(L_lo, L_hi, b) for b in ch) for ch in ch1)
    SLOT2 = max(sum(ct(L_a, L_b, b) for b in ch) for ch in ch2)

    nc = bacc.Bacc("TRN2", target_bir_lowering=False, debug=False,
                   num_devices=NCORES)

    hb_lo = nc.dram_tensor("hb_lo", [HALF, D1], BF16, kind="ExternalInput")
    hb_hi = nc.dram_tensor("hb_hi", [HALF, D1], BF16, kind="ExternalInput")
    w1 = nc.dram_tensor("w1", [IN_D, HID_D], BF16, kind="ExternalInput")
    w2 = nc.dram_tensor("w2", [HID_D, OUT_D], BF16, kind="ExternalInput")
    dn = nc.dram_tensor("dn", [128, PB], F32, kind="ExternalInput")
    sdn = nc.dram_tensor("sdn", [128, PB], F32, kind="ExternalInput")
    b1r = nc.dram_tensor("b1r", [128, D1], F32, kind="ExternalInput")
    b2r = nc.dram_tensor("b2r", [128, D2], F32, kind="ExternalInput")
    jrep = nc.dram_tensor("jrep", [128, LMAX * 128], BF16, kind="ExternalInput")
    ident = nc.dram_tensor("ident", [128, 128], BF16, kind="ExternalInput")
    gidx = nc.dram_tensor("gidx", [128, T1 * 8], I16, kind="ExternalInput")
    dstl = nc.dram_tensor("dstl", [128, max(T1L, 1)], BF16, kind="ExternalInput")
    gidx2 = nc.dram_tensor("gidx2", [128, T2 * 8], I16, kind="ExternalInput")
    dstl2 = nc.dram_tensor("dstl2", [128, max(T2L, 1)], BF16,
                           kind="ExternalInput")

    out_loc = nc.dram_tensor("out_loc", [PB * 128, D2], F32, kind="ExternalOutput")

    y2w_loc_a = nc.dram_tensor("y2w_loc_a", [SPLIT * 128, D2], BF16)
    y2w_loc_b = nc.dram_tensor("y2w_loc_b", [(PB - SPLIT) * 128, D2], BF16)
    y2w_full_a = nc.dram_tensor("y2w_full_a", [NCORES * SPLIT * 128, D2], BF16,
                                addr_space="Shared")
    y2w_full_b = nc.dram_tensor("y2w_full_b", [NCORES * (PB - SPLIT) * 128, D2],
                                BF16, addr_space="Shared")

    rg = [list(range(NCORES))]
    EQ = mybir.AluOpType.is_equal
    RELU = mybir.ActivationFunctionType.Relu
    COPY = mybir.ActivationFunctionType.Copy

    with tile.TileContext(nc) as tc:
        with (
            tc.tile_pool(name="persist", bufs=1) as pp,
            tc.tile_pool(name="sbuf", bufs=2) as sb,
            tc.tile_pool(name="post", bufs=2) as pq,
            tc.tile_pool(name="psA", bufs=2, space="PSUM") as psA,
            tc.tile_pool(name="psW", bufs=2, space="PSUM") as psW,
            tc.tile_pool(name="psT", bufs=2, space="PSUM") as psT,
            tc.tile_pool(name="psY", bufs=2, space="PSUM") as psY,
        ):
            # ---- persistent constants
            jr_t = pp.tile([128, LMAX * 128], BF16)
            nc.sync.dma_start(out=jr_t[:], in_=jrep[:])
            id_t = pp.tile([128, 128], BF16)
            nc.sync.dma_start(out=id_t[:], in_=ident[:])
            w1_t = pp.tile([IN_D, HID_D], BF16)
            nc.sync.dma_start(out=w1_t[:], in_=w1[:])
            w2_t = pp.tile([HID_D, OUT_D], BF16)
            nc.sync.dma_start(out=w2_t[:], in_=w2[:])
            d_t = pp.tile([128, PB], F32)
            nc.sync.dma_start(out=d_t[:], in_=dn[:])
            sd_t = pp.tile([128, PB], F32)
            nc.sync.dma_start(out=sd_t[:], in_=sdn[:])
            dstl_t = pp.tile([128, max(T1L, 1)], BF16)
            nc.sync.dma_start(out=dstl_t[:], in_=dstl[:])
            dstl2_t = pp.tile([128, max(T2L, 1)], BF16)
            nc.sync.dma_start(out=dstl2_t[:], in_=dstl2[:])
            if not b1z:
                b1_t = pp.tile([128, D1], F32)
                nc.sync.dma_start(out=b1_t[:], in_=b1r[:])
            if not b2z:
                b2_t = pp.tile([128, D2], F32)
                nc.sync.dma_start(out=b2_t[:], in_=b2r[:])

            def agg_matmuls(agg_ps, gt, oh, b, base0, base1, L0, L1, D):
                """Identity + one-hot accumulation for one block. gt layout
                per table: [M identity subtiles, L leftover]. Returns new
                (base0, base1)."""
                Ls = (L0[b], L1[b])
                tot = 2 * M + Ls[0] + Ls[1]
                k = 0
                lbase = 0
                for t, base in ((0, base0), (1, base1)):
                    for c in range(M):
                        nc.tensor.matmul(agg_ps[:, :D], lhsT=id_t[:],
                                         rhs=gt[:, base + c, :D],
                                         start=(k == 0), stop=(k == tot - 1))
                        k += 1
                    for c in range(Ls[t]):
                        nc.tensor.matmul(
                            agg_ps[:, :D],
                            lhsT=oh[:, (lbase + c) * 128:(lbase + c + 1) * 128],
                            rhs=gt[:, base + M + c, :D],
                            start=(k == 0), stop=(k == tot - 1))
                        k += 1
                    lbase += Ls[t]
                return base0 + M + Ls[0], base1 + M + Ls[1]

            # ---- layer 1
            goff = 0   # subtile offset into gidx (chunk order)
            doff = 0   # leftover-subtile offset into dstl (block order)
            for ch in ch1:
                CL = sum(M + L_lo[b] for b in ch)
                CH = sum(M + L_hi[b] for b in ch)
                CT = CL + CH
                gx = sb.tile([128, max(SLOT1, SLOT2) * 8], I16, tag="gx")
                nc.sync.dma_start(out=gx[:, :CT * 8],
                                  in_=gidx[:, goff * 8:(goff + CT) * 8])
                gt = sb.tile([128, SLOT1, D1], BF16, tag="gath")
                nc.gpsimd.dma_gather(
                    out_ap=gt[:, :CL, :], in_ap=hb_lo[:],
                    idxs_ap=gx[:, :CL * 8],
                    num_idxs=CL * 128, num_idxs_reg=CL * 128,
                    elem_size=D1, single_packet=False)
                nc.gpsimd.dma_gather(
                    out_ap=gt[:, CL:CT, :], in_ap=hb_hi[:],
                    idxs_ap=gx[:, CL * 8:CT * 8],
                    num_idxs=CH * 128, num_idxs_reg=CH * 128,
                    elem_size=D1, single_packet=False)
                base0 = 0
                base1 = CL
                for b in ch:
                    Lt = L_lo[b] + L_hi[b]
                    oh = sb.tile([128, LMAX * 128], BF16, tag="oh")
                    if Lt:
                        nc.vector.tensor_tensor(
                            out=oh[:, :Lt * 128],
                            in0=dstl_t[:, doff:doff + Lt].to_broadcast(
                                [128, Lt, 128]),
                            in1=jr_t[:, :Lt * 128], op=EQ)
                        doff += Lt
                    agg_ps = psA.tile([128, D1], F32, space="PSUM", tag="agg")
                    base0, base1 = agg_matmuls(agg_ps, gt, oh, b, base0, base1,
                                               L_lo, L_hi, D1)
                    # z = agg @ W1 (via PE transposes, per batch)
                    agg_sb = pq.tile([128, D1], BF16, tag="aggsb")
                    nc.scalar.copy(agg_sb[:], agg_ps[:])
                    zW_ps = psW.tile([128, D1], F32, space="PSUM", tag="zw")
                    for bb in range(B):
                        tr_ps = psT.tile([HID_D, 128], BF16, space="PSUM", tag="tr")
                        nc.tensor.transpose(
                            tr_ps[:], agg_sb[:, bb * HID_D:(bb + 1) * HID_D],
                            id_t[:])
                        tr_sb = pq.tile([HID_D, 128], BF16, tag="trsb")
                        nc.scalar.copy(tr_sb[:], tr_ps[:])
                        nc.tensor.matmul(
                            zW_ps[:, bb * HID_D:(bb + 1) * HID_D],
                            lhsT=tr_sb[:], rhs=w1_t[:], start=True, stop=True)
                    # y1 = relu(d*z + b1)
                    y1r = pq.tile([128, D1], BF16, tag="y1r")
                    if b1z:
                        nc.scalar.activation(y1r[:], zW_ps[:], RELU,
                                             scale=d_t[:, b:b + 1])
                    else:
                        t0 = pq.tile([128, D1], F32, tag="zb0")
                        nc.vector.tensor_scalar_mul(t0[:], zW_ps[:], d_t[:, b:b + 1])
                        t1 = pq.tile([128, D1], F32, tag="zb1")
                        nc.vector.tensor_tensor(out=t1[:], in0=t0[:], in1=b1_t[:],
                                                op=mybir.AluOpType.add)
                        nc.scalar.activation(y1r[:], t1[:], RELU)
                    # y2w row = (y1 * s) @ W2
                    y2w_ps = psY.tile([128, D2], F32, space="PSUM", tag="y2w")
                    for bb in range(B):
                        tr2_ps = psT.tile([HID_D, 128], BF16, space="PSUM", tag="tr")
                        nc.tensor.transpose(
                            tr2_ps[:], y1r[:, bb * HID_D:(bb + 1) * HID_D], id_t[:])
                        tr2_sb = pq.tile([HID_D, 128], BF16, tag="trsb")
                        nc.scalar.copy(tr2_sb[:], tr2_ps[:])
                        nc.tensor.matmul(
                            y2w_ps[:, bb * OUT_D:(bb + 1) * OUT_D],
                            lhsT=tr2_sb[:], rhs=w2_t[:], start=True, stop=True)
                    y2w_sb = pq.tile([128, D2], BF16, tag="y2wsb")
                    nc.scalar.activation(y2w_sb[:], y2w_ps[:], COPY,
                                         scale=sd_t[:, b:b + 1])
                    if b < SPLIT:
                        nc.sync.dma_start(
                            out=y2w_loc_a[b * 128:(b + 1) * 128, :], in_=y2w_sb[:])
                    else:
                        nc.sync.dma_start(
                            out=y2w_loc_b[(b - SPLIT) * 128:(b - SPLIT + 1) * 128, :],
                            in_=y2w_sb[:])
                    if b == SPLIT - 1 and upto == 'l2':
                        if collectives:
                            nc.gpsimd.collective_compute(
                                "AllGather", mybir.AluOpType.bypass,
                                replica_groups=rg,
                                ins=[y2w_loc_a[:]], outs=[y2w_full_a[:]])
                        else:
                            for c in range(NCORES):
                                nc.sync.dma_start(
                                    out=y2w_full_a[c * SPLIT * 128:(c + 1) * SPLIT * 128, :],
                                    in_=y2w_loc_a[:])
                goff += CT

            # ---- exchange second table half
            if upto == 'l2':
                if collectives:
                    nc.gpsimd.collective_compute(
                        "AllGather", mybir.AluOpType.bypass, replica_groups=rg,
                        ins=[y2w_loc_b[:]], outs=[y2w_full_b[:]])
                else:
                    nb128 = (PB - SPLIT) * 128
                    for c in range(NCORES):
                        nc.sync.dma_start(
                            out=y2w_full_b[c * nb128:(c + 1) * nb128, :],
                            in_=y2w_loc_b[:])

            # ---- layer 2
            goff = 0
            doff = 0
            for ch in (ch2 if upto == 'l2' else []):
                CA = sum(M + L_a[b] for b in ch)
                CB = sum(M + L_b[b] for b in ch)
                CT = CA + CB
                gx = sb.tile([128, max(SLOT1, SLOT2) * 8], I16, tag="gx")
                nc.sync.dma_start(out=gx[:, :CT * 8],
                                  in_=gidx2[:, goff * 8:(goff + CT) * 8])
                g2v = sb.tile([128, SLOT2, D2], BF16, tag="gath")
                nc.gpsimd.dma_gather(
                    out_ap=g2v[:, :CA, :], in_ap=y2w_full_a[:],
                    idxs_ap=gx[:, :CA * 8],
                    num_idxs=CA * 128, num_idxs_reg=CA * 128,
                    elem_size=D2, single_packet=False)
                nc.gpsimd.dma_gather(
                    out_ap=g2v[:, CA:CT, :], in_ap=y2w_full_b[:],
                    idxs_ap=gx[:, CA * 8:CT * 8],
                    num_idxs=CB * 128, num_idxs_reg=CB * 128,
                    elem_size=D2, single_packet=False)
                base0 = 0
                base1 = CA
                for b in ch:
                    Lt = L_a[b] + L_b[b]
                    oh = sb.tile([128, LMAX * 128], BF16, tag="oh")
                    if Lt:
                        nc.vector.tensor_tensor(
                            out=oh[:, :Lt * 128],
                            in0=dstl2_t[:, doff:doff + Lt].to_broadcast(
                                [128, Lt, 128]),
                            in1=jr_t[:, :Lt * 128], op=EQ)
                        doff += Lt
                    agg_ps = psA.tile([128, D1], F32, space="PSUM", tag="agg")
                    base0, base1 = agg_matmuls(agg_ps, g2v, oh, b, base0, base1,
                                               L_a, L_b, D2)
                    out_sb = pq.tile([128, D2], F32, tag="outsb")
                    if b2z:
                        nc.scalar.activation(out_sb[:], agg_ps[:, :D2], COPY,
                                             scale=d_t[:, b:b + 1])
                    else:
                        t0 = pq.tile([128, D2], F32, tag="ob0")
                        nc.vector.tensor_scalar_mul(t0[:], agg_ps[:, :D2],
                                                    d_t[:, b:b + 1])
                        nc.vector.tensor_tensor(out=out_sb[:], in0=t0[:],
                                                in1=b2_t[:], op=mybir.AluOpType.add)
                    nc.sync.dma_start(out=out_loc[b * 128:(b + 1) * 128, :],
                                      in_=out_sb[:])
                goff += CT

    nc.compile()
    return nc


# ------------------------------------------------------------------- driver

def _prepare_inputs(h, W1, b1, W2, b2, src, dst):
    percore, meta, s_pad, d_pad, slot_of = _preprocess(src, dst)
    meta["b1z"] = bool(np.all(np.asarray(b1) == 0))
    meta["b2z"] = bool(np.all(np.asarray(b2) == 0))

    # hB rows by slot: [slot, B*F], pre-scaled by s_norm, bf16
    hs = np.asarray(h, np.float32).transpose(1, 0, 2).reshape(N, B * IN_D)
    hb = np.zeros((NPAD, D1), np.float32)
    hb[slot_of[:N]] = hs
    hb *= s_pad[:, None].astype(np.float32)
    hb = hb.astype(NPBF16)

    LMAX = max(max(meta["L_lo"][b] + meta["L_hi"][b] for b in range(PB)),
               max(meta["L_a"][b] + meta["L_b"][b] for b in range(PB)), 1)
    jr = np.tile(np.arange(128, dtype=np.float32), (128, LMAX)).astype(NPBF16)
    idm = np.eye(128, dtype=np.float32).astype(NPBF16)

    d_all = d_pad.reshape(NCORES, PB, 128)
    s_all = s_pad.reshape(NCORES, PB, 128)

    common = {
        "hb_lo": hb[:HALF], "hb_hi": hb[HALF:],
        "w1": np.asarray(W1, np.float32).astype(NPBF16),
        "w2": np.asarray(W2, np.float32).astype(NPBF16),
        "b1r": np.tile(np.asarray(b1, np.float32), (128, B)),
        "b2r": np.tile(np.asarray(b2, np.float32), (128, B)),
        "jrep": jr, "ident": idm,
    }
    in_maps = []
    for c in range(NCORES):
        m = dict(common, **percore[c])
        m["dn"] = np.ascontiguousarray(d_all[c].T, dtype=np.float32)
        m["sdn"] = np.ascontiguousarray(s_all[c].T, dtype=np.float32)
        in_maps.append(m)
    return in_maps, meta, slot_of


_BUILD_CACHE = {}


def _get_nc(meta):
    key = tuple(sorted((k, tuple(v) if isinstance(v, list) else v)
                       for k, v in meta.items()))
    if key not in _BUILD_CACHE:
        nc = _build(meta)
        nc.m = get_hw_module(nc.m)
        _BUILD_CACHE[key] = nc
    return _BUILD_CACHE[key]


def _assemble(results, slot_of):
    full = np.concatenate([results[c]["out_loc"] for c in range(NCORES)], axis=0)
    out = full.reshape(NPAD, B, OUT_D).transpose(1, 0, 2)
    out = out[:, slot_of[:N], :]
    return np.ascontiguousarray(out, dtype=np.float32)


def kernel(h, W1, b1, W2, b2, src, dst):
    in_maps, meta, slot_of = _prepare_inputs(h, W1, b1, W2, b2, src, dst)
    nc = _get_nc(meta)
    res = run_bass_kernel_spmd(nc, in_maps, core_ids=list(range(NCORES)))
    return _assemble(res.results, slot_of)


# revision 8
# speedup vs baseline: 2.4304x; 1.1326x over previous
"""Trainium2 Bass kernel for a 2-layer GraphConv GCN (nn_GCNN_69776038691375).

reference semantics:
    x = h.swapaxes(0,1)                       # [N, B, F]
    out_deg/in_deg from src/dst, clipped at 1
    s = out_deg**-0.5 ; d = in_deg**-0.5
    layer(x, W, b) = (segsum((x*s)[src] -> dst) * d) @ W + b
    y = relu(layer(x, W1, b1)); out = layer(y, W2, b2); return out.swapaxes(0,1)

Design (v3):
  * Degree norms are topology-only -> computed on host (bincount), shipped as
    tiny per-node scale vectors. No on-device degree pass.
  * Layer-1 gathers read rows of hB = (x*s) directly (host-prescaled, bf16,
    512B rows) -- W1 is applied after aggregation per dst block.
  * Layer-2 gathers rows of y2w = (y1*s) @ W2 (bf16, 256B rows), exchanged
    via two AllGathers (the first fires early to overlap with layer 1).
  * dst-node sharding: core c owns blocks [c*49, (c+1)*49) of 128 nodes.
  * Hybrid aggregation: for each dst-local slot j, its first <=M edges (per
    src-table) are placed at partition j of "identity subtiles" -> the
    aggregation matmul uses a constant identity lhsT (no one-hot build).
    Overflow edges go to packed subtiles reduced with a one-hot built by
    is_equal vs iota (DVE). Empty identity slots gather a guaranteed-zero
    row: two nodes are host-swapped with pad slots so every gather table
    has a zero row (pads also get s=0 so their y2w rows vanish).
  * Gathers are chunked over several blocks per dma_gather call to amortize
    the SWDGE fixed descriptor-generation overhead on the Pool engine.
"""

import numpy as np
import ml_dtypes

import concourse.bacc as bacc
import concourse.bass as bass
import concourse.mybir as mybir
import concourse.tile as tile
from concourse.bass_interp import get_hw_module
from concourse.bass_utils import run_bass_kernel_spmd

F32 = mybir.dt.float32
BF16 = mybir.dt.bfloat16
I16 = mybir.dt.int16
NPBF16 = ml_dtypes.bfloat16

# problem sizes (hardcoded per contract)
N = 50000
E = 800000
B = 4
IN_D, HID_D, OUT_D = 64, 64, 32
NCORES = 8
PB = 49                 # blocks per core
NB = NCORES * PB        # 392 global blocks
NPAD = NB * 128         # 50176
HALF = NPAD // 2        # 25088: dma_gather int16 index limit split point
D1 = B * HID_D          # 256 bf16 per hB row (512B)
D2 = B * OUT_D          # 128 bf16 per y2w row (256B)
SENT = 250              # one-hot sentinel for padded edges
SPLIT = 24              # L1 block index after which the first y2w AllGather fires
G1 = 7                  # L1 blocks per gather chunk
G2 = 13                 # L2 blocks per gather chunk
M = 6                   # identity-subtile depth per (block, table)

# node<->slot permutation: slots 127 and 3199 become pads (zero rows for the
# lo / A / B gather tables); their nodes move to the tail pad slots. The hi
# table's zero row is the untouched pad slot 50000.
SWAPS = ((127, NPAD - 2), (3199, NPAD - 1))
Z_LO = 127
Z_HI = 50000 - HALF
Z_A = 127               # slot 127: block 0 < SPLIT, posA = 127
Z_B = 127               # slot 3199: block 24, posB = 127


def _chunks(g):
    return [list(range(i, min(i + g, PB))) for i in range(0, PB, g)]


# ---------------------------------------------------------------- host side

def _wrap_idx(flat):
    """dma_gather index layout: idx j of a gather lives at [j%16, j//16],
    replicated across the 8 groups of 16 partitions. flat: [T, 128] int16
    (subtile-major). Returns [128, T*8]."""
    T = flat.shape[0]
    w = flat.reshape(T, 8, 16).transpose(2, 0, 1).reshape(16, T * 8)
    return np.tile(w, (8, 1)).astype(np.int16)


def _place_block(j_arr, idx_arr, zidx):
    """Distribute one (core, block, table) edge slice.

    Each dst-local j gets its first <=M edges at partition j of identity
    subtiles 0..M-1 (empty slots -> zidx, a zero row). Returns
    (id_idx [M,128] int16, left_idx, left_j) for the overflow edges."""
    order = np.argsort(j_arr, kind="stable")
    j_s = j_arr[order]
    s_s = idx_arr[order]
    n = len(j_s)
    if n:
        newgrp = np.concatenate([[True], j_s[1:] != j_s[:-1]])
        gstart = np.maximum.accumulate(np.where(newgrp, np.arange(n), 0))
        rank = np.arange(n) - gstart
    else:
        rank = np.zeros(0, np.int64)
    idm = rank < M
    id_idx = np.full((M, 128), zidx, np.int16)
    id_idx[rank[idm], j_s[idm]] = s_s[idm]
    return id_idx, s_s[~idm], j_s[~idm]


def _preprocess(src, dst):
    src = np.asarray(src).astype(np.int64)
    dst = np.asarray(dst).astype(np.int64)

    # node -> slot permutation
    slot_of = np.arange(NPAD, dtype=np.int64)
    for a, b in SWAPS:
        slot_of[a], slot_of[b] = slot_of[b], slot_of[a]
    src = slot_of[src]
    dst = slot_of[dst]

    # degree norms by slot (topology only -> host). Pads: s=0 (kills their
    # y2w rows even with nonzero bias), d=1.
    s_pad = np.zeros(NPAD, np.float64)
    d_pad = np.ones(NPAD, np.float64)
    s_cnt = np.bincount(src, minlength=NPAD).astype(np.float64)
    d_cnt = np.bincount(dst, minlength=NPAD).astype(np.float64)
    real = np.zeros(NPAD, bool)
    real[slot_of[:N]] = True
    s_pad[real] = np.maximum(s_cnt[real], 1.0) ** -0.5
    d_pad[real] = np.maximum(d_cnt[real], 1.0) ** -0.5

    blk = dst >> 7
    dloc = dst & 127

    # L1 tables: lo/hi by src slot half; L2 tables: A/B by src block < SPLIT
    t1 = (src >= HALF).astype(np.int64)
    i1 = src - t1 * HALF
    src_c = src // (PB * 128)
    src_b = (src % (PB * 128)) >> 7
    src_p = src & 127
    t2 = (src_b >= SPLIT).astype(np.int64)
    i2 = np.where(t2 == 0,
                  src_c * (SPLIT * 128) + src_b * 128 + src_p,
                  src_c * ((PB - SPLIT) * 128) + (src_b - SPLIT) * 128 + src_p)

    def build(tt, ii, z0, z1, chunks):
        order = np.lexsort((ii, tt, blk))
        o_blk, o_t, o_i, o_j = blk[order], tt[order], ii[order], dloc[order]
        cnt = np.bincount(o_blk * 2 + o_t, minlength=NB * 2).reshape(NB, 2)
        starts = np.concatenate([[0], np.cumsum(cnt.ravel())])[:-1].reshape(NB, 2)
        id_idx = {}
        left = {}
        nleft = np.zeros((NB, 2), np.int64)
        for g in range(NB):
            for t in range(2):
                st, n = int(starts[g, t]), int(cnt[g, t])
                z = z0 if t == 0 else z1
                idt, li, lj = _place_block(o_j[st:st + n], o_i[st:st + n], z)
                id_idx[(g, t)] = idt
                left[(g, t)] = (li, lj)
                nleft[g, t] = len(li)
        Lsub = (-(-nleft // 128)).reshape(NCORES, PB, 2).max(axis=0)  # [PB, 2]
        L0, L1 = Lsub[:, 0].astype(int), Lsub[:, 1].astype(int)
        percore = []
        for c in range(NCORES):
            gs = []      # chunk-ordered gather subtiles
            ds = []      # block-ordered one-hot dst-locals (leftovers only)
            for ch in chunks:
                for t in range(2):
                    for b in ch:
                        g = c * PB + b
                        L = int((L0 if t == 0 else L1)[b])
                        gs.append(id_idx[(g, t)])
                        li, lj = left[(g, t)]
                        z = z0 if t == 0 else z1
                        gi = np.full(L * 128, z, np.int16)
                        gi[:len(li)] = li.astype(np.int16)
                        gs.append(gi.reshape(L, 128))
            for b in range(PB):
                for t in range(2):
                    g = c * PB + b
                    L = int((L0 if t == 0 else L1)[b])
                    li, lj = left[(g, t)]
                    dl = np.full(L * 128, SENT, np.int16)
                    dl[:len(lj)] = lj.astype(np.int16)
                    ds.append(dl.reshape(L, 128))
            gidx = _wrap_idx(np.concatenate(gs, axis=0))
            dstl = np.ascontiguousarray(
                np.concatenate(ds, axis=0).T).astype(NPBF16)
            percore.append((gidx, dstl))
        return percore, L0.tolist(), L1.tolist()

    pc1, L_lo, L_hi = build(t1, i1, Z_LO, Z_HI, _chunks(G1))
    pc2, L_a, L_b = build(t2, i2, Z_A, Z_B, _chunks(G2))

    percore = [{"gidx": pc1[c][0], "dstl": pc1[c][1],
                "gidx2": pc2[c][0], "dstl2": pc2[c][1]}
               for c in range(NCORES)]
    meta = dict(L_lo=L_lo, L_hi=L_hi, L_a=L_a, L_b=L_b)
    return percore, meta, s_pad, d_pad, slot_of


# -------------------------------------------------------------- bass program

def _jmax(meta):
    """Max per-chunk leftover subtiles (sizes the iota table / one-hot tile)."""
    L_lo, L_hi = meta["L_lo"], meta["L_hi"]
    L_a, L_b = meta["L_a"], meta["L_b"]
    j1 = max(sum(L_lo[b] + L_hi[b] for b in ch) for ch in _chunks(G1))
    j2 = max(sum(L_a[b] + L_b[b] for b in ch) for ch in _chunks(G2))
    return max(j1, j2, 1)


def _build(meta, collectives=True, upto='l2'):
    L_lo, L_hi = meta["L_lo"], meta["L_hi"]
    L_a, L_b = meta["L_a"], meta["L_b"]
    b1z, b2z = meta["b1z"], meta["b2z"]

    def ct(L0, L1, b):
        return 2 * M + L0[b] + L1[b]

    T1 = sum(ct(L_lo, L_hi, b) for b in range(PB))
    T2 = sum(ct(L_a, L_b, b) for b in range(PB))
    T1L = sum(L_lo) + sum(L_hi)
    T2L = sum(L_a) + sum(L_b)
    JMAX = _jmax(meta)
    ch1, ch2 = _chunks(G1), _chunks(G2)
    SLOT1 = max(sum(ct(L_lo, L_hi, b) for b in ch) for ch in ch1)
    SLOT2 = max(sum(ct(L_a, L_b, b) for b in ch) for ch in ch2)

    nc = bacc.Bacc("TRN2", target_bir_lowering=False, debug=False,
                   num_devices=NCORES)

    hb_lo = nc.dram_tensor("hb_lo", [HALF, D1], BF16, kind="ExternalInput")
    hb_hi = nc.dram_tensor("hb_hi", [HALF, D1], BF16, kind="ExternalInput")
    w1d = nc.dram_tensor("w1d", [128, 128], BF16, kind="ExternalInput")
    w2d = nc.dram_tensor("w2d", [128, 64], BF16, kind="ExternalInput")
    dn = nc.dram_tensor("dn", [128, PB], F32, kind="ExternalInput")
    sdn = nc.dram_tensor("sdn", [128, PB], F32, kind="ExternalInput")
    b1r = nc.dram_tensor("b1r", [128, D1], F32, kind="ExternalInput")
    b2r = nc.dram_tensor("b2r", [128, D2], F32, kind="ExternalInput")
    jrep = nc.dram_tensor("jrep", [128, JMAX * 128], BF16, kind="ExternalInput")
    ident = nc.dram_tensor("ident", [128, 128], BF16, kind="ExternalInput")
    gidx = nc.dram_tensor("gidx", [128, T1 * 8], I16, kind="ExternalInput")
    dstl = nc.dram_tensor("dstl", [128, max(T1L, 1)], BF16, kind="ExternalInput")
    gidx2 = nc.dram_tensor("gidx2", [128, T2 * 8], I16, kind="ExternalInput")
    dstl2 = nc.dram_tensor("dstl2", [128, max(T2L, 1)], BF16,
                           kind="ExternalInput")

    out_loc = nc.dram_tensor("out_loc", [PB * 128, D2], F32, kind="ExternalOutput")

    y2w_loc_a = nc.dram_tensor("y2w_loc_a", [SPLIT * 128, D2], BF16)
    y2w_loc_b = nc.dram_tensor("y2w_loc_b", [(PB - SPLIT) * 128, D2], BF16)
    y2w_full_a = nc.dram_tensor("y2w_full_a", [NCORES * SPLIT * 128, D2], BF16,
                                addr_space="Shared")
    y2w_full_b = nc.dram_tensor("y2w_full_b", [NCORES * (PB - SPLIT) * 128, D2],
                                BF16, addr_space="Shared")

    rg = [list(range(NCORES))]
    EQ = mybir.AluOpType.is_equal
    RELU = mybir.ActivationFunctionType.Relu
    COPY = mybir.ActivationFunctionType.Copy

    with tile.TileContext(nc) as tc:
        with (
            tc.tile_pool(name="persist", bufs=1) as pp,
            tc.tile_pool(name="sbuf", bufs=2) as sb,
            tc.tile_pool(name="gxp", bufs=2) as gxp,
            tc.tile_pool(name="post", bufs=3) as pq,
            tc.tile_pool(name="psA", bufs=3, space="PSUM") as psA,
            tc.tile_pool(name="psW", bufs=2, space="PSUM") as psW,
            tc.tile_pool(name="psT", bufs=2, space="PSUM") as psT,
            tc.tile_pool(name="psY", bufs=1, space="PSUM") as psY,
        ):
            # ---- persistent constants
            jr_t = pp.tile([128, JMAX * 128], BF16)
            nc.sync.dma_start(out=jr_t[:], in_=jrep[:])
            id_t = pp.tile([128, 128], BF16)
            nc.sync.dma_start(out=id_t[:], in_=ident[:])
            w1_t = pp.tile([128, 128], BF16)
            nc.sync.dma_start(out=w1_t[:], in_=w1d[:])
            w2_t = pp.tile([128, 64], BF16)
            nc.sync.dma_start(out=w2_t[:], in_=w2d[:])
            d_t = pp.tile([128, PB], F32)
            nc.sync.dma_start(out=d_t[:], in_=dn[:])
            sd_t = pp.tile([128, PB], F32)
            nc.sync.dma_start(out=sd_t[:], in_=sdn[:])
            dstl_t = pp.tile([128, max(T1L, 1)], BF16)
            nc.sync.dma_start(out=dstl_t[:], in_=dstl[:])
            dstl2_t = pp.tile([128, max(T2L, 1)], BF16)
            nc.sync.dma_start(out=dstl2_t[:], in_=dstl2[:])
            if not b1z:
                b1_t = pp.tile([128, D1], F32)
                nc.sync.dma_start(out=b1_t[:], in_=b1r[:])
            if not b2z:
                b2_t = pp.tile([128, D2], F32)
                nc.sync.dma_start(out=b2_t[:], in_=b2r[:])

            def agg_matmuls(agg_ps, gt, oh, lbase, b, base0, base1, L0, L1, D):
                """Identity + one-hot accumulation for one block. gt layout
                per table: [M identity subtiles, L leftover]."""
                Ls = (L0[b], L1[b])
                tot = 2 * M + Ls[0] + Ls[1]
                k = 0
                lb = lbase
                for t, base in ((0, base0), (1, base1)):
                    for c in range(M):
                        nc.tensor.matmul(agg_ps[:, :D], lhsT=id_t[:],
                                         rhs=gt[:, base + c, :D],
                                         start=(k == 0), stop=(k == tot - 1))
                        k += 1
                    for c in range(Ls[t]):
                        nc.tensor.matmul(
                            agg_ps[:, :D],
                            lhsT=oh[:, (lb + c) * 128:(lb + c + 1) * 128],
                            rhs=gt[:, base + M + c, :D],
                            start=(k == 0), stop=(k == tot - 1))
                        k += 1
                    lb += Ls[t]
                return base0 + M + Ls[0], base1 + M + Ls[1]

            def l1_tail(b, agg_ps):
                # z = agg @ W1 via paired transposes + block-diag weights
                agg_sb = pq.tile([128, D1], BF16, tag="aggsb")
                nc.scalar.copy(agg_sb[:], agg_ps[:])
                zW_ps = psW.tile([128, D1], F32, space="PSUM", tag="zw")
                for hf in range(2):
                    tr_ps = psT.tile([128, 128], BF16, space="PSUM", tag="tr")
                    nc.tensor.transpose(
                        tr_ps[:], agg_sb[:, hf * 128:(hf + 1) * 128], id_t[:])
                    tr_sb = pq.tile([128, 128], BF16, tag="trsb")
                    nc.scalar.copy(tr_sb[:], tr_ps[:])
                    nc.tensor.matmul(
                        zW_ps[:, hf * 128:(hf + 1) * 128],
                        lhsT=tr_sb[:], rhs=w1_t[:], start=True, stop=True)
                # y1 = relu(d*z + b1)
                y1r = pq.tile([128, D1], BF16, tag="y1r")
                if b1z:
                    nc.scalar.activation(y1r[:], zW_ps[:], RELU,
                                         scale=d_t[:, b:b + 1])
                else:
                    t0 = pq.tile([128, D1], F32, tag="zb0")
                    nc.vector.tensor_scalar_mul(t0[:], zW_ps[:], d_t[:, b:b + 1])
                    t1 = pq.tile([128, D1], F32, tag="zb1")
                    nc.vector.tensor_tensor(out=t1[:], in0=t0[:], in1=b1_t[:],
                                            op=mybir.AluOpType.add)
                    nc.scalar.activation(y1r[:], t1[:], RELU)
                # y2w row = (y1 * s) @ W2
                y2w_ps = psY.tile([128, D2], F32, space="PSUM", tag="y2w")
                for hf in range(2):
                    tr2_ps = psT.tile([128, 128], BF16, space="PSUM", tag="tr")
                    nc.tensor.transpose(
                        tr2_ps[:], y1r[:, hf * 128:(hf + 1) * 128], id_t[:])
                    tr2_sb = pq.tile([128, 128], BF16, tag="trsb")
                    nc.scalar.copy(tr2_sb[:], tr2_ps[:])
                    nc.tensor.matmul(
                        y2w_ps[:, hf * 64:(hf + 1) * 64],
                        lhsT=tr2_sb[:], rhs=w2_t[:], start=True, stop=True)
                y2w_sb = pq.tile([128, D2], BF16, tag="y2wsb")
                nc.scalar.activation(y2w_sb[:], y2w_ps[:], COPY,
                                     scale=sd_t[:, b:b + 1])
                if b < SPLIT:
                    nc.sync.dma_start(
                        out=y2w_loc_a[b * 128:(b + 1) * 128, :], in_=y2w_sb[:])
                else:
                    nc.sync.dma_start(
                        out=y2w_loc_b[(b - SPLIT) * 128:(b - SPLIT + 1) * 128, :],
                        in_=y2w_sb[:])
                if b == SPLIT - 1 and upto == 'l2':
                    if collectives:
                        nc.gpsimd.collective_compute(
                            "AllGather", mybir.AluOpType.bypass,
                            replica_groups=rg,
                            ins=[y2w_loc_a[:]], outs=[y2w_full_a[:]])
                    else:
                        for c in range(NCORES):
                            nc.sync.dma_start(
                                out=y2w_full_a[c * SPLIT * 128:(c + 1) * SPLIT * 128, :],
                                in_=y2w_loc_a[:])

            def l2_tail(b, agg_ps):
                out_sb = pq.tile([128, D2], F32, tag="outsb")
                if b2z:
                    nc.scalar.activation(out_sb[:], agg_ps[:, :D2], COPY,
                                         scale=d_t[:, b:b + 1])
                else:
                    t0 = pq.tile([128, D2], F32, tag="ob0")
                    nc.vector.tensor_scalar_mul(t0[:], agg_ps[:, :D2],
                                                d_t[:, b:b + 1])
                    nc.vector.tensor_tensor(out=out_sb[:], in0=t0[:],
                                            in1=b2_t[:], op=mybir.AluOpType.add)
                nc.sync.dma_start(out=out_loc[b * 128:(b + 1) * 128, :],
                                  in_=out_sb[:])

            # per-chunk issue: index load, gathers, one-hot build
            def issue_chunk(ci, chunks, L0, L1, gidx_d, dstl_sb, tabs, elem,
                            slot, goffs, doffs):
                ch = chunks[ci]
                C0 = sum(M + L0[b] for b in ch)
                C1 = sum(M + L1[b] for b in ch)
                CT = C0 + C1
                goff = goffs[ci]
                gx = gxp.tile([128, max(SLOT1, SLOT2) * 8], I16, tag="gx")
                nc.sync.dma_start(out=gx[:, :CT * 8],
                                  in_=gidx_d[:, goff * 8:(goff + CT) * 8])
                gt = sb.tile([128, slot, elem], BF16, tag="gath")
                nc.gpsimd.dma_gather(
                    out_ap=gt[:, :C0, :], in_ap=tabs[0][:],
                    idxs_ap=gx[:, :C0 * 8],
                    num_idxs=C0 * 128, num_idxs_reg=C0 * 128,
                    elem_size=elem, single_packet=False)
                nc.gpsimd.dma_gather(
                    out_ap=gt[:, C0:CT, :], in_ap=tabs[1][:],
                    idxs_ap=gx[:, C0 * 8:CT * 8],
                    num_idxs=C1 * 128, num_idxs_reg=C1 * 128,
                    elem_size=elem, single_packet=False)
                chL = sum(L0[b] + L1[b] for b in ch)
                oh = sb.tile([128, JMAX * 128], BF16, tag="oh")
                if chL:
                    doff = doffs[ci]
                    nc.vector.tensor_tensor(
                        out=oh[:, :chL * 128],
                        in0=dstl_sb[:, doff:doff + chL].to_broadcast(
                            [128, chL, 128]),
                        in1=jr_t[:, :chL * 128], op=EQ)
                return gt, oh, C0

            def run_layer(chunks, L0, L1, gidx_d, dstl_sb, tabs, elem, slot,
                          D, tail):
                goffs, doffs = [0], [0]
                for ch in chunks:
                    goffs.append(goffs[-1] + sum(ct(L0, L1, b) for b in ch))
                    doffs.append(doffs[-1] + sum(L0[b] + L1[b] for b in ch))
                pending = None
                state = issue_chunk(0, chunks, L0, L1, gidx_d, dstl_sb, tabs,
                                    elem, slot, goffs, doffs)
                for ci, ch in enumerate(chunks):
                    gt, oh, C0 = state
                    if ci + 1 < len(chunks):
                        state = issue_chunk(ci + 1, chunks, L0, L1, gidx_d,
                                            dstl_sb, tabs, elem, slot, goffs,
                                            doffs)
                    base0, base1 = 0, C0
                    lbase = 0
                    for b in ch:
                        agg_ps = psA.tile([128, D1], F32, space="PSUM", tag="agg")
                        base0, base1 = agg_matmuls(agg_ps, gt, oh, lbase, b,
                                                   base0, base1, L0, L1, D)
                        lbase += L0[b] + L1[b]
                        if pending is not None:
                            tail(*pending)
                        pending = (b, agg_ps)
                if pending is not None:
                    tail(*pending)

            # ---- layer 1
            run_layer(ch1, L_lo, L_hi, gidx, dstl_t, (hb_lo, hb_hi), D1, SLOT1,
                      D1, l1_tail)

            # ---- exchange second table half
            if upto == 'l2':
                if collectives:
                    nc.gpsimd.collective_compute(
                        "AllGather", mybir.AluOpType.bypass, replica_groups=rg,
                        ins=[y2w_loc_b[:]], outs=[y2w_full_b[:]])
                else:
                    nb128 = (PB - SPLIT) * 128
                    for c in range(NCORES):
                        nc.sync.dma_start(
                            out=y2w_full_b[c * nb128:(c + 1) * nb128, :],
                            in_=y2w_loc_b[:])

                # ---- layer 2
                run_layer(ch2, L_a, L_b, gidx2, dstl2_t,
                          (y2w_full_a, y2w_full_b), D2, SLOT2, D2, l2_tail)

    nc.compile()
    return nc


# ------------------------------------------------------------------- driver

def _prepare_inputs(h, W1, b1, W2, b2, src, dst):
    percore, meta, s_pad, d_pad, slot_of = _preprocess(src, dst)
    meta["b1z"] = bool(np.all(np.asarray(b1) == 0))
    meta["b2z"] = bool(np.all(np.asarray(b2) == 0))

    # hB rows by slot: [slot, B*F], pre-scaled by s_norm, bf16
    hs = np.asarray(h, np.float32).transpose(1, 0, 2).reshape(N, B * IN_D)
    hb = np.zeros((NPAD, D1), np.float32)
    hb[slot_of[:N]] = hs
    hb *= s_pad[:, None].astype(np.float32)
    hb = hb.astype(NPBF16)

    jr = np.tile(np.arange(128, dtype=np.float32),
                 (128, _jmax(meta))).astype(NPBF16)
    idm = np.eye(128, dtype=np.float32).astype(NPBF16)
    w1f = np.asarray(W1, np.float32)
    w2f = np.asarray(W2, np.float32)
    w1d = np.zeros((128, 128), np.float32)
    w1d[:64, :64] = w1f
    w1d[64:, 64:] = w1f
    w2d = np.zeros((128, 64), np.float32)
    w2d[:64, :32] = w2f
    w2d[64:, 32:] = w2f

    d_all = d_pad.reshape(NCORES, PB, 128)
    s_all = s_pad.reshape(NCORES, PB, 128)

    common = {
        "hb_lo": hb[:HALF], "hb_hi": hb[HALF:],
        "w1d": w1d.astype(NPBF16),
        "w2d": w2d.astype(NPBF16),
        "b1r": np.tile(np.asarray(b1, np.float32), (128, B)),
        "b2r": np.tile(np.asarray(b2, np.float32), (128, B)),
        "jrep": jr, "ident": idm,
    }
    in_maps = []
    for c in range(NCORES):
        m = dict(common, **percore[c])
        m["dn"] = np.ascontiguousarray(d_all[c].T, dtype=np.float32)
        m["sdn"] = np.ascontiguousarray(s_all[c].T, dtype=np.float32)
        in_maps.append(m)
    return in_maps, meta, slot_of


_BUILD_CACHE = {}


def _get_nc(meta):
    key = tuple(sorted((k, tuple(v) if isinstance(v, list) else v)
                       for k, v in meta.items()))
    if key not in _BUILD_CACHE:
        nc = _build(meta)
        nc.m = get_hw_module(nc.m)
        _BUILD_CACHE[key] = nc
    return _BUILD_CACHE[key]


def _assemble(results, slot_of):
    full = np.concatenate([results[c]["out_loc"] for c in range(NCORES)], axis=0)
    out = full.reshape(NPAD, B, OUT_D).transpose(1, 0, 2)
    out = out[:, slot_of[:N], :]
    return np.ascontiguousarray(out, dtype=np.float32)


def kernel(h, W1, b1, W2, b2, src, dst):
    in_maps, meta, slot_of = _prepare_inputs(h, W1, b1, W2, b2, src, dst)
    nc = _get_nc(meta)
    res = run_bass_kernel_spmd(nc, in_maps, core_ids=list(range(NCORES)))
    return _assemble(res.results, slot_of)


# revision 11
# speedup vs baseline: 2.6445x; 1.0881x over previous
"""Trainium2 Bass kernel for a 2-layer GraphConv GCN (nn_GCNN_69776038691375).

reference semantics:
    x = h.swapaxes(0,1)                       # [N, B, F]
    out_deg/in_deg from src/dst, clipped at 1
    s = out_deg**-0.5 ; d = in_deg**-0.5
    layer(x, W, b) = (segsum((x*s)[src] -> dst) * d) @ W + b
    y = relu(layer(x, W1, b1)); out = layer(y, W2, b2); return out.swapaxes(0,1)

Design (v3):
  * Degree norms are topology-only -> computed on host (bincount), shipped as
    tiny per-node scale vectors. No on-device degree pass.
  * Layer-1 gathers read rows of hB = (x*s) directly (host-prescaled, bf16,
    512B rows) -- W1 is applied after aggregation per dst block.
  * Layer-2 gathers rows of y2w = (y1*s) @ W2 (bf16, 256B rows), exchanged
    via two AllGathers (the first fires early to overlap with layer 1).
  * dst-node sharding: core c owns blocks [c*49, (c+1)*49) of 128 nodes.
  * Hybrid aggregation: for each dst-local slot j, its first <=M edges (per
    src-table) are placed at partition j of "identity subtiles" -> the
    aggregation matmul uses a constant identity lhsT (no one-hot build).
    Overflow edges go to packed subtiles reduced with a one-hot built by
    is_equal vs iota (DVE). Empty identity slots gather a guaranteed-zero
    row: two nodes are host-swapped with pad slots so every gather table
    has a zero row (pads also get s=0 so their y2w rows vanish).
  * Gathers are chunked over several blocks per dma_gather call to amortize
    the SWDGE fixed descriptor-generation overhead on the Pool engine.
"""

import numpy as np
import ml_dtypes

import concourse.bacc as bacc
import concourse.bass as bass
import concourse.mybir as mybir
import concourse.tile as tile
from concourse.bass_interp import get_hw_module
from concourse.bass_utils import run_bass_kernel_spmd

F32 = mybir.dt.float32
BF16 = mybir.dt.bfloat16
I16 = mybir.dt.int16
NPBF16 = ml_dtypes.bfloat16

# problem sizes (hardcoded per contract)
N = 50000
E = 800000
B = 4
IN_D, HID_D, OUT_D = 64, 64, 32
NCORES = 8
PB = 49                 # blocks per core
NB = NCORES * PB        # 392 global blocks
NPAD = NB * 128         # 50176
HALF = NPAD // 2        # 25088: dma_gather int16 index limit split point
D1 = B * HID_D          # 256 bf16 per hB row (512B)
D2 = B * OUT_D          # 128 bf16 per y2w row (256B)
SENT = 250              # one-hot sentinel for padded edges
SPLIT = 24              # L1 block index after which the first y2w AllGather fires
G1 = 4                  # L1 blocks per gather chunk
G2 = 8                  # L2 blocks per gather chunk
M = 6                   # identity-subtile depth per (block, table)

# node<->slot permutation: slots 127 and 3199 become pads (zero rows for the
# lo / A / B gather tables); their nodes move to the tail pad slots. The hi
# table's zero row is the untouched pad slot 50000.
SWAPS = ((127, NPAD - 2), (3199, NPAD - 1))
Z_LO = 127
Z_HI = 50000 - HALF
Z_A = 127               # slot 127: block 0 < SPLIT, posA = 127
Z_B = 127               # slot 3199: block 24, posB = 127


def _chunks(g):
    return [list(range(i, min(i + g, PB))) for i in range(0, PB, g)]


# ---------------------------------------------------------------- host side

def _wrap_idx(flat):
    """dma_gather index layout: idx j of a gather lives at [j%16, j//16],
    replicated across the 8 groups of 16 partitions. flat: [T, 128] int16
    (subtile-major). Returns [128, T*8]."""
    T = flat.shape[0]
    w = flat.reshape(T, 8, 16).transpose(2, 0, 1).reshape(16, T * 8)
    return np.tile(w, (8, 1)).astype(np.int16)


def _place_block(j_arr, idx_arr, zidx):
    """Distribute one (core, block, table) edge slice.

    Each dst-local j gets its first <=M edges at partition j of identity
    subtiles 0..M-1 (empty slots -> zidx, a zero row). Returns
    (id_idx [M,128] int16, left_idx, left_j) for the overflow edges."""
    order = np.argsort(j_arr, kind="stable")
    j_s = j_arr[order]
    s_s = idx_arr[order]
    n = len(j_s)
    if n:
        newgrp = np.concatenate([[True], j_s[1:] != j_s[:-1]])
        gstart = np.maximum.accumulate(np.where(newgrp, np.arange(n), 0))
        rank = np.arange(n) - gstart
    else:
        rank = np.zeros(0, np.int64)
    idm = rank < M
    id_idx = np.full((M, 128), zidx, np.int16)
    id_idx[rank[idm], j_s[idm]] = s_s[idm]
    return id_idx, s_s[~idm], j_s[~idm]


def _preprocess(src, dst):
    src = np.asarray(src).astype(np.int64)
    dst = np.asarray(dst).astype(np.int64)

    # node -> slot permutation
    slot_of = np.arange(NPAD, dtype=np.int64)
    for a, b in SWAPS:
        slot_of[a], slot_of[b] = slot_of[b], slot_of[a]
    src = slot_of[src]
    dst = slot_of[dst]

    # degree norms by slot (topology only -> host). Pads: s=0 (kills their
    # y2w rows even with nonzero bias), d=1.
    s_pad = np.zeros(NPAD, np.float64)
    d_pad = np.ones(NPAD, np.float64)
    s_cnt = np.bincount(src, minlength=NPAD).astype(np.float64)
    d_cnt = np.bincount(dst, minlength=NPAD).astype(np.float64)
    real = np.zeros(NPAD, bool)
    real[slot_of[:N]] = True
    s_pad[real] = np.maximum(s_cnt[real], 1.0) ** -0.5
    d_pad[real] = np.maximum(d_cnt[real], 1.0) ** -0.5

    blk = dst >> 7
    dloc = dst & 127

    # L1 tables: lo/hi by src slot half; L2 tables: A/B by src block < SPLIT
    t1 = (src >= HALF).astype(np.int64)
    i1 = src - t1 * HALF
    src_c = src // (PB * 128)
    src_b = (src % (PB * 128)) >> 7
    src_p = src & 127
    t2 = (src_b >= SPLIT).astype(np.int64)
    i2 = np.where(t2 == 0,
                  src_c * (SPLIT * 128) + src_b * 128 + src_p,
                  src_c * ((PB - SPLIT) * 128) + (src_b - SPLIT) * 128 + src_p)

    def build(tt, ii, z0, z1, chunks):
        order = np.lexsort((ii, tt, blk))
        o_blk, o_t, o_i, o_j = blk[order], tt[order], ii[order], dloc[order]
        cnt = np.bincount(o_blk * 2 + o_t, minlength=NB * 2).reshape(NB, 2)
        starts = np.concatenate([[0], np.cumsum(cnt.ravel())])[:-1].reshape(NB, 2)
        id_idx = {}
        left = {}
        nleft = np.zeros((NB, 2), np.int64)
        for g in range(NB):
            for t in range(2):
                st, n = int(starts[g, t]), int(cnt[g, t])
                z = z0 if t == 0 else z1
                idt, li, lj = _place_block(o_j[st:st + n], o_i[st:st + n], z)
                id_idx[(g, t)] = idt
                left[(g, t)] = (li, lj)
                nleft[g, t] = len(li)
        Lsub = (-(-nleft // 128)).reshape(NCORES, PB, 2).max(axis=0)  # [PB, 2]
        L0, L1 = Lsub[:, 0].astype(int), Lsub[:, 1].astype(int)
        percore = []
        for c in range(NCORES):
            gs = []      # chunk-ordered gather subtiles
            ds = []      # block-ordered one-hot dst-locals (leftovers only)
            for ch in chunks:
                for t in range(2):
                    for b in ch:
                        g = c * PB + b
                        L = int((L0 if t == 0 else L1)[b])
                        gs.append(id_idx[(g, t)])
                        li, lj = left[(g, t)]
                        z = z0 if t == 0 else z1
                        gi = np.full(L * 128, z, np.int16)
                        gi[:len(li)] = li.astype(np.int16)
                        gs.append(gi.reshape(L, 128))
            for b in range(PB):
                for t in range(2):
                    g = c * PB + b
                    L = int((L0 if t == 0 else L1)[b])
                    li, lj = left[(g, t)]
                    dl = np.full(L * 128, SENT, np.int16)
                    dl[:len(lj)] = lj.astype(np.int16)
                    ds.append(dl.reshape(L, 128))
            gidx = _wrap_idx(np.concatenate(gs, axis=0))
            dstl = np.ascontiguousarray(
                np.concatenate(ds, axis=0).T).astype(NPBF16)
            percore.append((gidx, dstl))
        return percore, L0.tolist(), L1.tolist()

    pc1, L_lo, L_hi = build(t1, i1, Z_LO, Z_HI, _chunks(G1))
    pc2, L_a, L_b = build(t2, i2, Z_A, Z_B, _chunks(G2))

    percore = [{"gidx": pc1[c][0], "dstl": pc1[c][1],
                "gidx2": pc2[c][0], "dstl2": pc2[c][1]}
               for c in range(NCORES)]
    meta = dict(L_lo=L_lo, L_hi=L_hi, L_a=L_a, L_b=L_b)
    return percore, meta, s_pad, d_pad, slot_of


# -------------------------------------------------------------- bass program

def _jmax(meta):
    """Max per-chunk leftover subtiles (sizes the iota table / one-hot tile)."""
    L_lo, L_hi = meta["L_lo"], meta["L_hi"]
    L_a, L_b = meta["L_a"], meta["L_b"]
    j1 = max(sum(L_lo[b] + L_hi[b] for b in ch) for ch in _chunks(G1))
    j2 = max(sum(L_a[b] + L_b[b] for b in ch) for ch in _chunks(G2))
    return max(j1, j2, 1)


def _build(meta, collectives=True, upto='l2'):
    L_lo, L_hi = meta["L_lo"], meta["L_hi"]
    L_a, L_b = meta["L_a"], meta["L_b"]
    b1z, b2z = meta["b1z"], meta["b2z"]

    def ct(L0, L1, b):
        return 2 * M + L0[b] + L1[b]

    T1 = sum(ct(L_lo, L_hi, b) for b in range(PB))
    T2 = sum(ct(L_a, L_b, b) for b in range(PB))
    T1L = sum(L_lo) + sum(L_hi)
    T2L = sum(L_a) + sum(L_b)
    JMAX = _jmax(meta)
    ch1, ch2 = _chunks(G1), _chunks(G2)
    SLOT1 = max(sum(ct(L_lo, L_hi, b) for b in ch) for ch in ch1)
    SLOT2 = max(sum(ct(L_a, L_b, b) for b in ch) for ch in ch2)

    nc = bacc.Bacc("TRN2", target_bir_lowering=False, debug=False,
                   num_devices=NCORES)

    hb_lo = nc.dram_tensor("hb_lo", [HALF, D1], BF16, kind="ExternalInput")
    hb_hi = nc.dram_tensor("hb_hi", [HALF, D1], BF16, kind="ExternalInput")
    w1d = nc.dram_tensor("w1d", [128, 128], BF16, kind="ExternalInput")
    w2d = nc.dram_tensor("w2d", [128, 64], BF16, kind="ExternalInput")
    dn = nc.dram_tensor("dn", [128, PB], F32, kind="ExternalInput")
    sdn = nc.dram_tensor("sdn", [128, PB], F32, kind="ExternalInput")
    b1r = nc.dram_tensor("b1r", [128, D1], F32, kind="ExternalInput")
    b2r = nc.dram_tensor("b2r", [128, D2], F32, kind="ExternalInput")
    jrep = nc.dram_tensor("jrep", [128, JMAX * 128], BF16, kind="ExternalInput")
    ident = nc.dram_tensor("ident", [128, 128], BF16, kind="ExternalInput")
    gidx = nc.dram_tensor("gidx", [128, T1 * 8], I16, kind="ExternalInput")
    dstl = nc.dram_tensor("dstl", [128, max(T1L, 1)], BF16, kind="ExternalInput")
    gidx2 = nc.dram_tensor("gidx2", [128, T2 * 8], I16, kind="ExternalInput")
    dstl2 = nc.dram_tensor("dstl2", [128, max(T2L, 1)], BF16,
                           kind="ExternalInput")

    out_loc = nc.dram_tensor("out_loc", [PB * 128, D2], F32, kind="ExternalOutput")

    y2w_loc_a = nc.dram_tensor("y2w_loc_a", [SPLIT * 128, D2], BF16)
    y2w_loc_b = nc.dram_tensor("y2w_loc_b", [(PB - SPLIT) * 128, D2], BF16)
    y2w_full_a = nc.dram_tensor("y2w_full_a", [NCORES * SPLIT * 128, D2], BF16,
                                addr_space="Shared")
    y2w_full_b = nc.dram_tensor("y2w_full_b", [NCORES * (PB - SPLIT) * 128, D2],
                                BF16, addr_space="Shared")

    rg = [list(range(NCORES))]
    EQ = mybir.AluOpType.is_equal
    RELU = mybir.ActivationFunctionType.Relu
    COPY = mybir.ActivationFunctionType.Copy

    with tile.TileContext(nc) as tc:
        with (
            tc.tile_pool(name="persist", bufs=1) as pp,
            tc.tile_pool(name="sbuf", bufs=3) as sb,
            tc.tile_pool(name="gxp", bufs=3) as gxp,
            tc.tile_pool(name="ohp", bufs=2) as ohp,
            tc.tile_pool(name="post", bufs=3) as pq,
            tc.tile_pool(name="psA", bufs=3, space="PSUM") as psA,
            tc.tile_pool(name="psW", bufs=2, space="PSUM") as psW,
            tc.tile_pool(name="psT", bufs=2, space="PSUM") as psT,
            tc.tile_pool(name="psY", bufs=1, space="PSUM") as psY,
        ):
            # ---- persistent constants
            jr_t = pp.tile([128, JMAX * 128], BF16)
            nc.sync.dma_start(out=jr_t[:], in_=jrep[:])
            id_t = pp.tile([128, 128], BF16)
            nc.sync.dma_start(out=id_t[:], in_=ident[:])
            w1_t = pp.tile([128, 128], BF16)
            nc.sync.dma_start(out=w1_t[:], in_=w1d[:])
            w2_t = pp.tile([128, 64], BF16)
            nc.sync.dma_start(out=w2_t[:], in_=w2d[:])
            d_t = pp.tile([128, PB], F32)
            nc.sync.dma_start(out=d_t[:], in_=dn[:])
            sd_t = pp.tile([128, PB], F32)
            nc.sync.dma_start(out=sd_t[:], in_=sdn[:])
            dstl_t = pp.tile([128, max(T1L, 1)], BF16)
            nc.sync.dma_start(out=dstl_t[:], in_=dstl[:])
            dstl2_t = pp.tile([128, max(T2L, 1)], BF16)
            nc.sync.dma_start(out=dstl2_t[:], in_=dstl2[:])
            if not b1z:
                b1_t = pp.tile([128, D1], F32)
                nc.sync.dma_start(out=b1_t[:], in_=b1r[:])
            if not b2z:
                b2_t = pp.tile([128, D2], F32)
                nc.sync.dma_start(out=b2_t[:], in_=b2r[:])

            def agg_matmuls(agg_ps, gt, oh, lbase, b, base0, base1, L0, L1, D):
                """Identity + one-hot accumulation for one block. gt layout
                per table: [M identity subtiles, L leftover]."""
                Ls = (L0[b], L1[b])
                tot = 2 * M + Ls[0] + Ls[1]
                k = 0
                lb = lbase
                for t, base in ((0, base0), (1, base1)):
                    for c in range(M):
                        nc.tensor.matmul(agg_ps[:, :D], lhsT=id_t[:],
                                         rhs=gt[:, base + c, :D],
                                         start=(k == 0), stop=(k == tot - 1))
                        k += 1
                    for c in range(Ls[t]):
                        nc.tensor.matmul(
                            agg_ps[:, :D],
                            lhsT=oh[:, (lb + c) * 128:(lb + c + 1) * 128],
                            rhs=gt[:, base + M + c, :D],
                            start=(k == 0), stop=(k == tot - 1))
                        k += 1
                    lb += Ls[t]
                return base0 + M + Ls[0], base1 + M + Ls[1]

            def l1_tail(b, agg_ps):
                # z = agg @ W1 via paired transposes + block-diag weights
                agg_sb = pq.tile([128, D1], BF16, tag="aggsb")
                nc.scalar.copy(agg_sb[:], agg_ps[:])
                zW_ps = psW.tile([128, D1], F32, space="PSUM", tag="zw")
                for hf in range(2):
                    tr_ps = psT.tile([128, 128], BF16, space="PSUM", tag="tr")
                    nc.tensor.transpose(
                        tr_ps[:], agg_sb[:, hf * 128:(hf + 1) * 128], id_t[:])
                    tr_sb = pq.tile([128, 128], BF16, tag="trsb")
                    nc.scalar.copy(tr_sb[:], tr_ps[:])
                    nc.tensor.matmul(
                        zW_ps[:, hf * 128:(hf + 1) * 128],
                        lhsT=tr_sb[:], rhs=w1_t[:], start=True, stop=True)
                # y1 = relu(d*z + b1)
                y1r = pq.tile([128, D1], BF16, tag="y1r")
                if b1z:
                    nc.scalar.activation(y1r[:], zW_ps[:], RELU,
                                         scale=d_t[:, b:b + 1])
                else:
                    t0 = pq.tile([128, D1], F32, tag="zb0")
                    nc.vector.tensor_scalar_mul(t0[:], zW_ps[:], d_t[:, b:b + 1])
                    t1 = pq.tile([128, D1], F32, tag="zb1")
                    nc.vector.tensor_tensor(out=t1[:], in0=t0[:], in1=b1_t[:],
                                            op=mybir.AluOpType.add)
                    nc.scalar.activation(y1r[:], t1[:], RELU)
                # y2w row = (y1 * s) @ W2
                y2w_ps = psY.tile([128, D2], F32, space="PSUM", tag="y2w")
                for hf in range(2):
                    tr2_ps = psT.tile([128, 128], BF16, space="PSUM", tag="tr")
                    nc.tensor.transpose(
                        tr2_ps[:], y1r[:, hf * 128:(hf + 1) * 128], id_t[:])
                    tr2_sb = pq.tile([128, 128], BF16, tag="trsb")
                    nc.scalar.copy(tr2_sb[:], tr2_ps[:])
                    nc.tensor.matmul(
                        y2w_ps[:, hf * 64:(hf + 1) * 64],
                        lhsT=tr2_sb[:], rhs=w2_t[:], start=True, stop=True)
                y2w_sb = pq.tile([128, D2], BF16, tag="y2wsb")
                nc.scalar.activation(y2w_sb[:], y2w_ps[:], COPY,
                                     scale=sd_t[:, b:b + 1])
                if b < SPLIT:
                    nc.sync.dma_start(
                        out=y2w_loc_a[b * 128:(b + 1) * 128, :], in_=y2w_sb[:])
                else:
                    nc.sync.dma_start(
                        out=y2w_loc_b[(b - SPLIT) * 128:(b - SPLIT + 1) * 128, :],
                        in_=y2w_sb[:])
                if b == SPLIT - 1 and upto == 'l2':
                    if collectives:
                        nc.gpsimd.collective_compute(
                            "AllGather", mybir.AluOpType.bypass,
                            replica_groups=rg,
                            ins=[y2w_loc_a[:]], outs=[y2w_full_a[:]])
                    else:
                        for c in range(NCORES):
                            nc.scalar.dma_start(
                                out=y2w_full_a[c * SPLIT * 128:(c + 1) * SPLIT * 128, :],
                                in_=y2w_loc_a[:])

            def l2_tail(b, agg_ps):
                out_sb = pq.tile([128, D2], F32, tag="outsb")
                if b2z:
                    nc.scalar.activation(out_sb[:], agg_ps[:, :D2], COPY,
                                         scale=d_t[:, b:b + 1])
                else:
                    t0 = pq.tile([128, D2], F32, tag="ob0")
                    nc.vector.tensor_scalar_mul(t0[:], agg_ps[:, :D2],
                                                d_t[:, b:b + 1])
                    nc.vector.tensor_tensor(out=out_sb[:], in0=t0[:],
                                            in1=b2_t[:], op=mybir.AluOpType.add)
                nc.sync.dma_start(out=out_loc[b * 128:(b + 1) * 128, :],
                                  in_=out_sb[:])

            # per-chunk issue: index load, gathers, one-hot build
            def issue_chunk(ci, chunks, L0, L1, gidx_d, dstl_sb, tabs, elem,
                            slot, goffs, doffs):
                ch = chunks[ci]
                C0 = sum(M + L0[b] for b in ch)
                C1 = sum(M + L1[b] for b in ch)
                CT = C0 + C1
                goff = goffs[ci]
                gx = gxp.tile([128, max(SLOT1, SLOT2) * 8], I16, tag="gx")
                nc.sync.dma_start(out=gx[:, :CT * 8],
                                  in_=gidx_d[:, goff * 8:(goff + CT) * 8])
                gt = sb.tile([128, slot, elem], BF16, tag="gath")
                nc.gpsimd.dma_gather(
                    out_ap=gt[:, :C0, :], in_ap=tabs[0][:],
                    idxs_ap=gx[:, :C0 * 8],
                    num_idxs=C0 * 128, num_idxs_reg=C0 * 128,
                    elem_size=elem, single_packet=False)
                nc.gpsimd.dma_gather(
                    out_ap=gt[:, C0:CT, :], in_ap=tabs[1][:],
                    idxs_ap=gx[:, C0 * 8:CT * 8],
                    num_idxs=C1 * 128, num_idxs_reg=C1 * 128,
                    elem_size=elem, single_packet=False)
                chL = sum(L0[b] + L1[b] for b in ch)
                oh = ohp.tile([128, JMAX * 128], BF16, tag="oh")
                if chL:
                    doff = doffs[ci]
                    nc.vector.tensor_tensor(
                        out=oh[:, :chL * 128],
                        in0=dstl_sb[:, doff:doff + chL].to_broadcast(
                            [128, chL, 128]),
                        in1=jr_t[:, :chL * 128], op=EQ)
                return gt, oh, C0

            def run_layer(chunks, L0, L1, gidx_d, dstl_sb, tabs, elem, slot,
                          D, tail):
                goffs, doffs = [0], [0]
                for ch in chunks:
                    goffs.append(goffs[-1] + sum(ct(L0, L1, b) for b in ch))
                    doffs.append(doffs[-1] + sum(L0[b] + L1[b] for b in ch))
                pending = None
                state = issue_chunk(0, chunks, L0, L1, gidx_d, dstl_sb, tabs,
                                    elem, slot, goffs, doffs)
                for ci, ch in enumerate(chunks):
                    gt, oh, C0 = state
                    if ci + 1 < len(chunks):
                        state = issue_chunk(ci + 1, chunks, L0, L1, gidx_d,
                                            dstl_sb, tabs, elem, slot, goffs,
                                            doffs)
                    base0, base1 = 0, C0
                    lbase = 0
                    for b in ch:
                        agg_ps = psA.tile([128, D1], F32, space="PSUM", tag="agg")
                        base0, base1 = agg_matmuls(agg_ps, gt, oh, lbase, b,
                                                   base0, base1, L0, L1, D)
                        lbase += L0[b] + L1[b]
                        if pending is not None:
                            tail(*pending)
                        pending = (b, agg_ps)
                if pending is not None:
                    tail(*pending)

            # ---- layer 1
            run_layer(ch1, L_lo, L_hi, gidx, dstl_t, (hb_lo, hb_hi), D1, SLOT1,
                      D1, l1_tail)

            # ---- exchange second table half
            if upto == 'l2':
                if collectives:
                    nc.gpsimd.collective_compute(
                        "AllGather", mybir.AluOpType.bypass, replica_groups=rg,
                        ins=[y2w_loc_b[:]], outs=[y2w_full_b[:]])
                else:
                    nb128 = (PB - SPLIT) * 128
                    for c in range(NCORES):
                        nc.scalar.dma_start(
                            out=y2w_full_b[c * nb128:(c + 1) * nb128, :],
                            in_=y2w_loc_b[:])

                # ---- layer 2
                run_layer(ch2, L_a, L_b, gidx2, dstl2_t,
                          (y2w_full_a, y2w_full_b), D2, SLOT2, D2, l2_tail)

    nc.compile()
    return nc


# ------------------------------------------------------------------- driver

def _prepare_inputs(h, W1, b1, W2, b2, src, dst):
    percore, meta, s_pad, d_pad, slot_of = _preprocess(src, dst)
    meta["b1z"] = bool(np.all(np.asarray(b1) == 0))
    meta["b2z"] = bool(np.all(np.asarray(b2) == 0))

    # hB rows by slot: [slot, B*F], pre-scaled by s_norm, bf16
    hs = np.asarray(h, np.float32).transpose(1, 0, 2).reshape(N, B * IN_D)
    hb = np.zeros((NPAD, D1), np.float32)
    hb[slot_of[:N]] = hs
    hb *= s_pad[:, None].astype(np.float32)
    hb = hb.astype(NPBF16)

    jr = np.tile(np.arange(128, dtype=np.float32),
                 (128, _jmax(meta))).astype(NPBF16)
    idm = np.eye(128, dtype=np.float32).astype(NPBF16)
    w1f = np.asarray(W1, np.float32)
    w2f = np.asarray(W2, np.float32)
    w1d = np.zeros((128, 128), np.float32)
    w1d[:64, :64] = w1f
    w1d[64:, 64:] = w1f
    w2d = np.zeros((128, 64), np.float32)
    w2d[:64, :32] = w2f
    w2d[64:, 32:] = w2f

    d_all = d_pad.reshape(NCORES, PB, 128)
    s_all = s_pad.reshape(NCORES, PB, 128)

    common = {
        "hb_lo": hb[:HALF], "hb_hi": hb[HALF:],
        "w1d": w1d.astype(NPBF16),
        "w2d": w2d.astype(NPBF16),
        "b1r": np.tile(np.asarray(b1, np.float32), (128, B)),
        "b2r": np.tile(np.asarray(b2, np.float32), (128, B)),
        "jrep": jr, "ident": idm,
    }
    in_maps = []
    for c in range(NCORES):
        m = dict(common, **percore[c])
        m["dn"] = np.ascontiguousarray(d_all[c].T, dtype=np.float32)
        m["sdn"] = np.ascontiguousarray(s_all[c].T, dtype=np.float32)
        in_maps.append(m)
    return in_maps, meta, slot_of


_BUILD_CACHE = {}


def _get_nc(meta):
    key = tuple(sorted((k, tuple(v) if isinstance(v, list) else v)
                       for k, v in meta.items()))
    if key not in _BUILD_CACHE:
        nc = _build(meta)
        nc.m = get_hw_module(nc.m)
        _BUILD_CACHE[key] = nc
    return _BUILD_CACHE[key]


def _assemble(results, slot_of):
    full = np.concatenate([results[c]["out_loc"] for c in range(NCORES)], axis=0)
    out = full.reshape(NPAD, B, OUT_D).transpose(1, 0, 2)
    out = out[:, slot_of[:N], :]
    return np.ascontiguousarray(out, dtype=np.float32)


def kernel(h, W1, b1, W2, b2, src, dst):
    in_maps, meta, slot_of = _prepare_inputs(h, W1, b1, W2, b2, src, dst)
    nc = _get_nc(meta)
    res = run_bass_kernel_spmd(nc, in_maps, core_ids=list(range(NCORES)))
    return _assemble(res.results, slot_of)


# revision 14
# speedup vs baseline: 2.6648x; 1.0077x over previous
"""Trainium2 Bass kernel for a 2-layer GraphConv GCN (nn_GCNN_69776038691375).

reference semantics:
    x = h.swapaxes(0,1)                       # [N, B, F]
    out_deg/in_deg from src/dst, clipped at 1
    s = out_deg**-0.5 ; d = in_deg**-0.5
    layer(x, W, b) = (segsum((x*s)[src] -> dst) * d) @ W + b
    y = relu(layer(x, W1, b1)); out = layer(y, W2, b2); return out.swapaxes(0,1)

Design (v3):
  * Degree norms are topology-only -> computed on host (bincount), shipped as
    tiny per-node scale vectors. No on-device degree pass.
  * Layer-1 gathers read rows of hB = (x*s) directly (host-prescaled, bf16,
    512B rows) -- W1 is applied after aggregation per dst block.
  * Layer-2 gathers rows of y2w = (y1*s) @ W2 (bf16, 256B rows), exchanged
    via two AllGathers (the first fires early to overlap with layer 1).
  * dst-node sharding: core c owns blocks [c*49, (c+1)*49) of 128 nodes.
  * Hybrid aggregation: for each dst-local slot j, its first <=M edges (per
    src-table) are placed at partition j of "identity subtiles" -> the
    aggregation matmul uses a constant identity lhsT (no one-hot build).
    Overflow edges go to packed subtiles reduced with a one-hot built by
    is_equal vs iota (DVE). Empty identity slots gather a guaranteed-zero
    row: two nodes are host-swapped with pad slots so every gather table
    has a zero row (pads also get s=0 so their y2w rows vanish).
  * Gathers are chunked over several blocks per dma_gather call to amortize
    the SWDGE fixed descriptor-generation overhead on the Pool engine.
"""

import numpy as np
import ml_dtypes

import concourse.bacc as bacc
import concourse.bass as bass
import concourse.mybir as mybir
import concourse.tile as tile
from concourse.bass_interp import get_hw_module
from concourse.bass_utils import run_bass_kernel_spmd

F32 = mybir.dt.float32
BF16 = mybir.dt.bfloat16
I16 = mybir.dt.int16
NPBF16 = ml_dtypes.bfloat16

# problem sizes (hardcoded per contract)
N = 50000
E = 800000
B = 4
IN_D, HID_D, OUT_D = 64, 64, 32
NCORES = 8
PB = 49                 # blocks per core
NB = NCORES * PB        # 392 global blocks
NPAD = NB * 128         # 50176
HALF = NPAD // 2        # 25088: dma_gather int16 index limit split point
D1 = B * HID_D          # 256 bf16 per hB row (512B)
D2 = B * OUT_D          # 128 bf16 per y2w row (256B)
SENT = 250              # one-hot sentinel for padded edges
SPLIT = 32              # L1 block index after which the first y2w AllGather fires
G1 = 4                  # L1 blocks per gather chunk
G2 = 8                  # L2 blocks per gather chunk
M = 6                   # identity-subtile depth per (block, table)

# node<->slot permutation: slots 127 and 3199 become pads (zero rows for the
# lo / A / B gather tables); their nodes move to the tail pad slots. The hi
# table's zero row is the untouched pad slot 50000.
SWAPS = ((127, NPAD - 2), (SPLIT * 128 + 127, NPAD - 1))
Z_LO = 127
Z_HI = 50000 - HALF
Z_A = 127               # slot 127: block 0 < SPLIT, posA = 127
Z_B = 127               # slot SPLIT*128+127: first B block, posB = 127


def _chunks(g):
    return [list(range(i, min(i + g, PB))) for i in range(0, PB, g)]


# ---------------------------------------------------------------- host side

def _wrap_idx(flat):
    """dma_gather index layout: idx j of a gather lives at [j%16, j//16],
    replicated across the 8 groups of 16 partitions. flat: [T, 128] int16
    (subtile-major). Returns [128, T*8]."""
    T = flat.shape[0]
    w = flat.reshape(T, 8, 16).transpose(2, 0, 1).reshape(16, T * 8)
    return np.tile(w, (8, 1)).astype(np.int16)


def _place_block(j_arr, idx_arr, zidx):
    """Distribute one (core, block, table) edge slice.

    Each dst-local j gets its first <=M edges at partition j of identity
    subtiles 0..M-1 (empty slots -> zidx, a zero row). Returns
    (id_idx [M,128] int16, left_idx, left_j) for the overflow edges."""
    order = np.argsort(j_arr, kind="stable")
    j_s = j_arr[order]
    s_s = idx_arr[order]
    n = len(j_s)
    if n:
        newgrp = np.concatenate([[True], j_s[1:] != j_s[:-1]])
        gstart = np.maximum.accumulate(np.where(newgrp, np.arange(n), 0))
        rank = np.arange(n) - gstart
    else:
        rank = np.zeros(0, np.int64)
    idm = rank < M
    id_idx = np.full((M, 128), zidx, np.int16)
    id_idx[rank[idm], j_s[idm]] = s_s[idm]
    return id_idx, s_s[~idm], j_s[~idm]


def _preprocess(src, dst):
    src = np.asarray(src).astype(np.int64)
    dst = np.asarray(dst).astype(np.int64)

    # node -> slot permutation
    slot_of = np.arange(NPAD, dtype=np.int64)
    for a, b in SWAPS:
        slot_of[a], slot_of[b] = slot_of[b], slot_of[a]
    src = slot_of[src]
    dst = slot_of[dst]

    # degree norms by slot (topology only -> host). Pads: s=0 (kills their
    # y2w rows even with nonzero bias), d=1.
    s_pad = np.zeros(NPAD, np.float64)
    d_pad = np.ones(NPAD, np.float64)
    s_cnt = np.bincount(src, minlength=NPAD).astype(np.float64)
    d_cnt = np.bincount(dst, minlength=NPAD).astype(np.float64)
    real = np.zeros(NPAD, bool)
    real[slot_of[:N]] = True
    s_pad[real] = np.maximum(s_cnt[real], 1.0) ** -0.5
    d_pad[real] = np.maximum(d_cnt[real], 1.0) ** -0.5

    blk = dst >> 7
    dloc = dst & 127

    # L1 tables: lo/hi by src slot half; L2 tables: A/B by src block < SPLIT
    t1 = (src >= HALF).astype(np.int64)
    i1 = src - t1 * HALF
    src_c = src // (PB * 128)
    src_b = (src % (PB * 128)) >> 7
    src_p = src & 127
    t2 = (src_b >= SPLIT).astype(np.int64)
    i2 = np.where(t2 == 0,
                  src_c * (SPLIT * 128) + src_b * 128 + src_p,
                  src_c * ((PB - SPLIT) * 128) + (src_b - SPLIT) * 128 + src_p)

    def build(tt, ii, z0, z1, chunks):
        order = np.lexsort((ii, tt, blk))
        o_blk, o_t, o_i, o_j = blk[order], tt[order], ii[order], dloc[order]
        cnt = np.bincount(o_blk * 2 + o_t, minlength=NB * 2).reshape(NB, 2)
        starts = np.concatenate([[0], np.cumsum(cnt.ravel())])[:-1].reshape(NB, 2)
        id_idx = {}
        left = {}
        nleft = np.zeros((NB, 2), np.int64)
        for g in range(NB):
            for t in range(2):
                st, n = int(starts[g, t]), int(cnt[g, t])
                z = z0 if t == 0 else z1
                idt, li, lj = _place_block(o_j[st:st + n], o_i[st:st + n], z)
                id_idx[(g, t)] = idt
                left[(g, t)] = (li, lj)
                nleft[g, t] = len(li)
        Lsub = (-(-nleft // 128)).reshape(NCORES, PB, 2).max(axis=0)  # [PB, 2]
        L0, L1 = Lsub[:, 0].astype(int), Lsub[:, 1].astype(int)
        percore = []
        for c in range(NCORES):
            gs = []      # chunk-ordered gather subtiles
            ds = []      # block-ordered one-hot dst-locals (leftovers only)
            for ch in chunks:
                for t in range(2):
                    for b in ch:
                        g = c * PB + b
                        L = int((L0 if t == 0 else L1)[b])
                        gs.append(id_idx[(g, t)])
                        li, lj = left[(g, t)]
                        z = z0 if t == 0 else z1
                        gi = np.full(L * 128, z, np.int16)
                        gi[:len(li)] = li.astype(np.int16)
                        gs.append(gi.reshape(L, 128))
            for b in range(PB):
                for t in range(2):
                    g = c * PB + b
                    L = int((L0 if t == 0 else L1)[b])
                    li, lj = left[(g, t)]
                    dl = np.full(L * 128, SENT, np.int16)
                    dl[:len(lj)] = lj.astype(np.int16)
                    ds.append(dl.reshape(L, 128))
            gidx = _wrap_idx(np.concatenate(gs, axis=0))
            dstl = np.ascontiguousarray(
                np.concatenate(ds, axis=0).T).astype(NPBF16)
            percore.append((gidx, dstl))
        return percore, L0.tolist(), L1.tolist()

    pc1, L_lo, L_hi = build(t1, i1, Z_LO, Z_HI, _chunks(G1))
    pc2, L_a, L_b = build(t2, i2, Z_A, Z_B, _chunks(G2))

    percore = [{"gidx": pc1[c][0], "dstl": pc1[c][1],
                "gidx2": pc2[c][0], "dstl2": pc2[c][1]}
               for c in range(NCORES)]
    meta = dict(L_lo=L_lo, L_hi=L_hi, L_a=L_a, L_b=L_b)
    return percore, meta, s_pad, d_pad, slot_of


# -------------------------------------------------------------- bass program

def _jmax(meta):
    """Max per-chunk leftover subtiles (sizes the iota table / one-hot tile)."""
    L_lo, L_hi = meta["L_lo"], meta["L_hi"]
    L_a, L_b = meta["L_a"], meta["L_b"]
    j1 = max(sum(L_lo[b] + L_hi[b] for b in ch) for ch in _chunks(G1))
    j2 = max(sum(L_a[b] + L_b[b] for b in ch) for ch in _chunks(G2))
    return max(j1, j2, 1)


def _build(meta, collectives=True, upto='l2'):
    L_lo, L_hi = meta["L_lo"], meta["L_hi"]
    L_a, L_b = meta["L_a"], meta["L_b"]
    b1z, b2z = meta["b1z"], meta["b2z"]

    def ct(L0, L1, b):
        return 2 * M + L0[b] + L1[b]

    T1 = sum(ct(L_lo, L_hi, b) for b in range(PB))
    T2 = sum(ct(L_a, L_b, b) for b in range(PB))
    T1L = sum(L_lo) + sum(L_hi)
    T2L = sum(L_a) + sum(L_b)
    JMAX = _jmax(meta)
    ch1, ch2 = _chunks(G1), _chunks(G2)
    SLOT1 = max(sum(ct(L_lo, L_hi, b) for b in ch) for ch in ch1)
    SLOT2 = max(sum(ct(L_a, L_b, b) for b in ch) for ch in ch2)

    nc = bacc.Bacc("TRN2", target_bir_lowering=False, debug=False,
                   num_devices=NCORES)

    hb_lo = nc.dram_tensor("hb_lo", [HALF, D1], BF16, kind="ExternalInput")
    hb_hi = nc.dram_tensor("hb_hi", [HALF, D1], BF16, kind="ExternalInput")
    w1d = nc.dram_tensor("w1d", [128, 128], BF16, kind="ExternalInput")
    w2d = nc.dram_tensor("w2d", [128, 64], BF16, kind="ExternalInput")
    dn = nc.dram_tensor("dn", [128, PB], F32, kind="ExternalInput")
    sdn = nc.dram_tensor("sdn", [128, PB], F32, kind="ExternalInput")
    b1r = nc.dram_tensor("b1r", [128, D1], F32, kind="ExternalInput")
    b2r = nc.dram_tensor("b2r", [128, D2], F32, kind="ExternalInput")
    jrep = nc.dram_tensor("jrep", [128, JMAX * 128], BF16, kind="ExternalInput")
    ident = nc.dram_tensor("ident", [128, 128], BF16, kind="ExternalInput")
    gidx = nc.dram_tensor("gidx", [128, T1 * 8], I16, kind="ExternalInput")
    dstl = nc.dram_tensor("dstl", [128, max(T1L, 1)], BF16, kind="ExternalInput")
    gidx2 = nc.dram_tensor("gidx2", [128, T2 * 8], I16, kind="ExternalInput")
    dstl2 = nc.dram_tensor("dstl2", [128, max(T2L, 1)], BF16,
                           kind="ExternalInput")

    out_loc = nc.dram_tensor("out_loc", [PB * 128, D2], F32, kind="ExternalOutput")

    y2w_loc_a = nc.dram_tensor("y2w_loc_a", [SPLIT * 128, D2], BF16)
    y2w_loc_b = nc.dram_tensor("y2w_loc_b", [(PB - SPLIT) * 128, D2], BF16)
    y2w_full_a = nc.dram_tensor("y2w_full_a", [NCORES * SPLIT * 128, D2], BF16,
                                addr_space="Shared")
    y2w_full_b = nc.dram_tensor("y2w_full_b", [NCORES * (PB - SPLIT) * 128, D2],
                                BF16, addr_space="Shared")

    rg = [list(range(NCORES))]
    EQ = mybir.AluOpType.is_equal
    RELU = mybir.ActivationFunctionType.Relu
    COPY = mybir.ActivationFunctionType.Copy

    with tile.TileContext(nc) as tc:
        with (
            tc.tile_pool(name="persist", bufs=1) as pp,
            tc.tile_pool(name="sbuf", bufs=3) as sb,
            tc.tile_pool(name="gxp", bufs=3) as gxp,
            tc.tile_pool(name="ohp", bufs=2) as ohp,
            tc.tile_pool(name="post", bufs=3) as pq,
            tc.tile_pool(name="psA", bufs=3, space="PSUM") as psA,
            tc.tile_pool(name="psW", bufs=2, space="PSUM") as psW,
            tc.tile_pool(name="psT", bufs=2, space="PSUM") as psT,
            tc.tile_pool(name="psY", bufs=1, space="PSUM") as psY,
        ):
            # ---- persistent constants (Activation DGE queue: keeps the SP
            # queue free so the first chunk's index load goes out first)
            jr_t = pp.tile([128, JMAX * 128], BF16)
            nc.scalar.dma_start(out=jr_t[:], in_=jrep[:])
            id_t = pp.tile([128, 128], BF16)
            nc.scalar.dma_start(out=id_t[:], in_=ident[:])
            w1_t = pp.tile([128, 128], BF16)
            nc.scalar.dma_start(out=w1_t[:], in_=w1d[:])
            w2_t = pp.tile([128, 64], BF16)
            nc.scalar.dma_start(out=w2_t[:], in_=w2d[:])
            d_t = pp.tile([128, PB], F32)
            nc.scalar.dma_start(out=d_t[:], in_=dn[:])
            sd_t = pp.tile([128, PB], F32)
            nc.scalar.dma_start(out=sd_t[:], in_=sdn[:])
            dstl_t = pp.tile([128, max(T1L, 1)], BF16)
            nc.scalar.dma_start(out=dstl_t[:], in_=dstl[:])
            dstl2_t = pp.tile([128, max(T2L, 1)], BF16)
            nc.scalar.dma_start(out=dstl2_t[:], in_=dstl2[:])
            if not b1z:
                b1_t = pp.tile([128, D1], F32)
                nc.scalar.dma_start(out=b1_t[:], in_=b1r[:])
            if not b2z:
                b2_t = pp.tile([128, D2], F32)
                nc.scalar.dma_start(out=b2_t[:], in_=b2r[:])

            def agg_matmuls(agg_ps, gt, oh, lbase, b, base0, base1, L0, L1, D):
                """Identity + one-hot accumulation for one block. gt layout
                per table: [M identity subtiles, L leftover]."""
                Ls = (L0[b], L1[b])
                tot = 2 * M + Ls[0] + Ls[1]
                k = 0
                lb = lbase
                for t, base in ((0, base0), (1, base1)):
                    for c in range(M):
                        nc.tensor.matmul(agg_ps[:, :D], lhsT=id_t[:],
                                         rhs=gt[:, base + c, :D],
                                         start=(k == 0), stop=(k == tot - 1))
                        k += 1
                    for c in range(Ls[t]):
                        nc.tensor.matmul(
                            agg_ps[:, :D],
                            lhsT=oh[:, (lb + c) * 128:(lb + c + 1) * 128],
                            rhs=gt[:, base + M + c, :D],
                            start=(k == 0), stop=(k == tot - 1))
                        k += 1
                    lb += Ls[t]
                return base0 + M + Ls[0], base1 + M + Ls[1]

            def l1_tail(b, agg_ps):
                # z = agg @ W1 via paired transposes + block-diag weights
                agg_sb = pq.tile([128, D1], BF16, tag="aggsb")
                nc.scalar.copy(agg_sb[:], agg_ps[:])
                zW_ps = psW.tile([128, D1], F32, space="PSUM", tag="zw")
                for hf in range(2):
                    tr_ps = psT.tile([128, 128], BF16, space="PSUM", tag="tr")
                    nc.tensor.transpose(
                        tr_ps[:], agg_sb[:, hf * 128:(hf + 1) * 128], id_t[:])
                    tr_sb = pq.tile([128, 128], BF16, tag="trsb")
                    nc.scalar.copy(tr_sb[:], tr_ps[:])
                    nc.tensor.matmul(
                        zW_ps[:, hf * 128:(hf + 1) * 128],
                        lhsT=tr_sb[:], rhs=w1_t[:], start=True, stop=True)
                # y1 = relu(d*z + b1)
                y1r = pq.tile([128, D1], BF16, tag="y1r")
                if b1z:
                    nc.scalar.activation(y1r[:], zW_ps[:], RELU,
                                         scale=d_t[:, b:b + 1])
                else:
                    t0 = pq.tile([128, D1], F32, tag="zb0")
                    nc.vector.tensor_scalar_mul(t0[:], zW_ps[:], d_t[:, b:b + 1])
                    t1 = pq.tile([128, D1], F32, tag="zb1")
                    nc.vector.tensor_tensor(out=t1[:], in0=t0[:], in1=b1_t[:],
                                            op=mybir.AluOpType.add)
                    nc.scalar.activation(y1r[:], t1[:], RELU)
                # y2w row = (y1 * s) @ W2
                y2w_ps = psY.tile([128, D2], F32, space="PSUM", tag="y2w")
                for hf in range(2):
                    tr2_ps = psT.tile([128, 128], BF16, space="PSUM", tag="tr")
                    nc.tensor.transpose(
                        tr2_ps[:], y1r[:, hf * 128:(hf + 1) * 128], id_t[:])
                    tr2_sb = pq.tile([128, 128], BF16, tag="trsb")
                    nc.scalar.copy(tr2_sb[:], tr2_ps[:])
                    nc.tensor.matmul(
                        y2w_ps[:, hf * 64:(hf + 1) * 64],
                        lhsT=tr2_sb[:], rhs=w2_t[:], start=True, stop=True)
                y2w_sb = pq.tile([128, D2], BF16, tag="y2wsb")
                nc.scalar.activation(y2w_sb[:], y2w_ps[:], COPY,
                                     scale=sd_t[:, b:b + 1])
                if b < SPLIT:
                    nc.sync.dma_start(
                        out=y2w_loc_a[b * 128:(b + 1) * 128, :], in_=y2w_sb[:])
                else:
                    nc.sync.dma_start(
                        out=y2w_loc_b[(b - SPLIT) * 128:(b - SPLIT + 1) * 128, :],
                        in_=y2w_sb[:])
                if b == SPLIT - 1 and upto == 'l2':
                    if collectives:
                        nc.gpsimd.collective_compute(
                            "AllGather", mybir.AluOpType.bypass,
                            replica_groups=rg,
                            ins=[y2w_loc_a[:]], outs=[y2w_full_a[:]])
                    else:
                        for c in range(NCORES):
                            nc.scalar.dma_start(
                                out=y2w_full_a[c * SPLIT * 128:(c + 1) * SPLIT * 128, :],
                                in_=y2w_loc_a[:])

            def l2_tail(b, agg_ps):
                out_sb = pq.tile([128, D2], F32, tag="outsb")
                if b2z:
                    nc.scalar.activation(out_sb[:], agg_ps[:, :D2], COPY,
                                         scale=d_t[:, b:b + 1])
                else:
                    t0 = pq.tile([128, D2], F32, tag="ob0")
                    nc.vector.tensor_scalar_mul(t0[:], agg_ps[:, :D2],
                                                d_t[:, b:b + 1])
                    nc.vector.tensor_tensor(out=out_sb[:], in0=t0[:],
                                            in1=b2_t[:], op=mybir.AluOpType.add)
                nc.sync.dma_start(out=out_loc[b * 128:(b + 1) * 128, :],
                                  in_=out_sb[:])

            # per-chunk machinery: index load, gathers, one-hot build
            GXSLOT = max(SLOT1, SLOT2)

            def layer_ctx(chunks, L0, L1, gidx_d, dstl_sb, tabs, elem, slot):
                goffs, doffs = [0], [0]
                for ch in chunks:
                    goffs.append(goffs[-1] + sum(ct(L0, L1, b) for b in ch))
                    doffs.append(doffs[-1] + sum(L0[b] + L1[b] for b in ch))
                ctx = dict(chunks=chunks, L0=L0, L1=L1, goffs=goffs,
                           doffs=doffs, gts={}, gxs={}, ohs={}, elem=elem)

                def gx_load(ci):
                    CT = goffs[ci + 1] - goffs[ci]
                    gx = gxp.tile([128, GXSLOT * 8], I16, tag="gx", name="gx")
                    nc.sync.dma_start(
                        out=gx[:, :CT * 8],
                        in_=gidx_d[:, goffs[ci] * 8:(goffs[ci] + CT) * 8])
                    ctx['gxs'][ci] = gx

                def gather(ci, t):
                    ch = chunks[ci]
                    C0 = sum(M + L0[b] for b in ch)
                    C1 = sum(M + L1[b] for b in ch)
                    if ci not in ctx['gts']:
                        ctx['gts'][ci] = sb.tile([128, slot, elem], BF16,
                                                 tag="gath", name="gt")
                    gt = ctx['gts'][ci]
                    gx = ctx['gxs'][ci]
                    if t == 0:
                        nc.gpsimd.dma_gather(
                            out_ap=gt[:, :C0, :], in_ap=tabs[0][:],
                            idxs_ap=gx[:, :C0 * 8],
                            num_idxs=C0 * 128, num_idxs_reg=C0 * 128,
                            elem_size=elem, single_packet=False)
                    else:
                        nc.gpsimd.dma_gather(
                            out_ap=gt[:, C0:C0 + C1, :], in_ap=tabs[1][:],
                            idxs_ap=gx[:, C0 * 8:(C0 + C1) * 8],
                            num_idxs=C1 * 128, num_idxs_reg=C1 * 128,
                            elem_size=elem, single_packet=False)

                def mk_oh(ci):
                    chL = doffs[ci + 1] - doffs[ci]
                    oh = ohp.tile([128, JMAX * 128], BF16, tag="oh", name="oh")
                    if chL:
                        doff = doffs[ci]
                        nc.vector.tensor_tensor(
                            out=oh[:, :chL * 128],
                            in0=dstl_sb[:, doff:doff + chL].to_broadcast(
                                [128, chL, 128]),
                            in1=jr_t[:, :chL * 128], op=EQ)
                    ctx['ohs'][ci] = oh

                ctx['gx_load'] = gx_load
                ctx['gather'] = gather
                ctx['mk_oh'] = mk_oh
                return ctx

            def run_layer(ctx, D, tail, stagger=False, preissued=False):
                chunks = ctx['chunks']
                L0, L1 = ctx['L0'], ctx['L1']
                n = len(chunks)
                if stagger:
                    # table-1 gathers wait on the second AllGather; issue them
                    # one chunk behind so they don't head-block the Pool queue
                    ctx['gx_load'](0)
                    ctx['gather'](0, 0)
                    ctx['mk_oh'](0)
                    if n > 1:
                        ctx['gx_load'](1)
                        ctx['gather'](1, 0)
                        ctx['mk_oh'](1)
                    ctx['gather'](0, 1)
                elif not preissued:
                    ctx['gx_load'](0)
                    ctx['gather'](0, 0)
                    ctx['gather'](0, 1)
                    ctx['mk_oh'](0)
                pending = None
                for ci, ch in enumerate(chunks):
                    if stagger:
                        if ci + 2 < n:
                            ctx['gx_load'](ci + 2)
                            ctx['gather'](ci + 2, 0)
                            ctx['mk_oh'](ci + 2)
                        if ci + 1 < n:
                            ctx['gather'](ci + 1, 1)
                    else:
                        if ci + 1 < n:
                            ctx['gx_load'](ci + 1)
                            ctx['gather'](ci + 1, 0)
                            ctx['gather'](ci + 1, 1)
                            ctx['mk_oh'](ci + 1)
                    gt = ctx['gts'].pop(ci)
                    oh = ctx['ohs'].pop(ci)
                    base0 = 0
                    base1 = sum(M + L0[b] for b in ch)
                    lbase = 0
                    for b in ch:
                        agg_ps = psA.tile([128, D1], F32, space="PSUM", tag="agg")
                        base0, base1 = agg_matmuls(agg_ps, gt, oh, lbase, b,
                                                   base0, base1, L0, L1, D)
                        lbase += L0[b] + L1[b]
                        if pending is not None:
                            tail(*pending)
                        pending = (b, agg_ps)
                if pending is not None:
                    tail(*pending)

            # ---- layer 1
            ctx1 = layer_ctx(ch1, L_lo, L_hi, gidx, dstl_t,
                             (hb_lo, hb_hi), D1, SLOT1)
            run_layer(ctx1, D1, l1_tail)

            # ---- exchange second table half
            if upto == 'l2':
                if collectives:
                    nc.gpsimd.collective_compute(
                        "AllGather", mybir.AluOpType.bypass, replica_groups=rg,
                        ins=[y2w_loc_b[:]], outs=[y2w_full_b[:]])
                else:
                    nb128 = (PB - SPLIT) * 128
                    for c in range(NCORES):
                        nc.scalar.dma_start(
                            out=y2w_full_b[c * nb128:(c + 1) * nb128, :],
                            in_=y2w_loc_b[:])

                # ---- layer 2 (staggered: table-B gathers lag one chunk)
                ctx2 = layer_ctx(ch2, L_a, L_b, gidx2, dstl2_t,
                                 (y2w_full_a, y2w_full_b), D2, SLOT2)
                run_layer(ctx2, D2, l2_tail, stagger=True)

    nc.compile()
    return nc


# ------------------------------------------------------------------- driver

def _prepare_inputs(h, W1, b1, W2, b2, src, dst):
    percore, meta, s_pad, d_pad, slot_of = _preprocess(src, dst)
    meta["b1z"] = bool(np.all(np.asarray(b1) == 0))
    meta["b2z"] = bool(np.all(np.asarray(b2) == 0))

    # hB rows by slot: [slot, B*F], pre-scaled by s_norm, bf16
    hs = np.asarray(h, np.float32).transpose(1, 0, 2).reshape(N, B * IN_D)
    hb = np.zeros((NPAD, D1), np.float32)
    hb[slot_of[:N]] = hs
    hb *= s_pad[:, None].astype(np.float32)
    hb = hb.astype(NPBF16)

    jr = np.tile(np.arange(128, dtype=np.float32),
                 (128, _jmax(meta))).astype(NPBF16)
    idm = np.eye(128, dtype=np.float32).astype(NPBF16)
    w1f = np.asarray(W1, np.float32)
    w2f = np.asarray(W2, np.float32)
    w1d = np.zeros((128, 128), np.float32)
    w1d[:64, :64] = w1f
    w1d[64:, 64:] = w1f
    w2d = np.zeros((128, 64), np.float32)
    w2d[:64, :32] = w2f
    w2d[64:, 32:] = w2f

    d_all = d_pad.reshape(NCORES, PB, 128)
    s_all = s_pad.reshape(NCORES, PB, 128)

    common = {
        "hb_lo": hb[:HALF], "hb_hi": hb[HALF:],
        "w1d": w1d.astype(NPBF16),
        "w2d": w2d.astype(NPBF16),
        "b1r": np.tile(np.asarray(b1, np.float32), (128, B)),
        "b2r": np.tile(np.asarray(b2, np.float32), (128, B)),
        "jrep": jr, "ident": idm,
    }
    in_maps = []
    for c in range(NCORES):
        m = dict(common, **percore[c])
        m["dn"] = np.ascontiguousarray(d_all[c].T, dtype=np.float32)
        m["sdn"] = np.ascontiguousarray(s_all[c].T, dtype=np.float32)
        in_maps.append(m)
    return in_maps, meta, slot_of


_BUILD_CACHE = {}


def _get_nc(meta):
    key = tuple(sorted((k, tuple(v) if isinstance(v, list) else v)
                       for k, v in meta.items()))
    if key not in _BUILD_CACHE:
        nc = _build(meta)
        nc.m = get_hw_module(nc.m)
        _BUILD_CACHE[key] = nc
    return _BUILD_CACHE[key]


def _assemble(results, slot_of):
    full = np.concatenate([results[c]["out_loc"] for c in range(NCORES)], axis=0)
    out = full.reshape(NPAD, B, OUT_D).transpose(1, 0, 2)
    out = out[:, slot_of[:N], :]
    return np.ascontiguousarray(out, dtype=np.float32)


def kernel(h, W1, b1, W2, b2, src, dst):
    in_maps, meta, slot_of = _prepare_inputs(h, W1, b1, W2, b2, src, dst)
    nc = _get_nc(meta)
    res = run_bass_kernel_spmd(nc, in_maps, core_ids=list(range(NCORES)))
    return _assemble(res.results, slot_of)
